# revision 36
# baseline (speedup 1.0000x reference)
"""Trainium2 Bass kernel for nn_ExchangeBlock (GNN message passing / e3nn-style
tensor-product edge block), SPMD across 8 NeuronCores.

Sharding: edges across the 8 cores; node features and params replicated.

v4 design notes:
- Edges are host-sorted into 4 parity classes (src&1, dst&1) so the pair-row
  parity select becomes a compile-time AP slice: no masks, no predicated
  copies, no gpsimd copy traffic at all.
- ONE dma_gather per block: the node table is bf16 pair rows (512B) with the
  fp32 position bit-packed into units 120:126 of each row, so geometry and
  the tensor product share the same gathered tile.  cell[graph_batch[src]]
  is a host-side index prep (like the baseline's graph_batch[src]) and is
  streamed per edge together with edge_shift.
- Blocks run in groups of 8 with two ScalarE activation-table phases per
  group (exp set: RBF; silu set: MLPs).  The per-edge scalar chain (Newton
  rsqrt, cutoff envelope as an exact degree-6 polynomial in d^2, RBF, demb)
  is batched once per group, so phase A is a handful of wide ops and the
  Square/Exp are single instructions the OoO scheduler cannot shred.
- The TP runs as outer-product features P[e,1344] built on DVE (bf16), PE
  transposes of P chunks (one full PSUM bank per 8 chunks), and accumulated
  128x128 matmuls against pre-scaled flattened weights.  The 1o/2e paths
  build m-major product tiles and reduce with 2 contiguous bf16 adds
  instead of a slow innermost-3 reduce.
- LN stats via one PSUM reduce + a batched ScalarE Square + one bf16
  reduce; the final w2 contraction fuses multiply+reduce into one
  scalar_tensor_tensor with accum_out.
- All MLP biases in this problem are exactly zero; _prep detects that and
  compiles the bias-free variant (rank-1 PE bias matmuls otherwise).
"""

import sys

sys.path.insert(0, "/opt/trn_rl_repo")

import numpy as np
import ml_dtypes

import concourse.bass as bass
import concourse.mybir as mybir
import concourse.tile as tile
from concourse import bacc
from concourse.bass_utils import run_bass_kernel_spmd
from concourse.masks import make_identity

F32 = mybir.dt.float32
BF16 = mybir.dt.bfloat16
I32 = mybir.dt.int32
I16 = mybir.dt.int16
AF = mybir.ActivationFunctionType
OP = mybir.AluOpType

# Problem constants
L0, L1, L2 = 32, 16, 8
NS = 128
NB = 64
CUTOFF = 7.0
N_NODES = 50000
N_EDGES = 400000
NODE_DIM = 120
NCORES = 8

BLK = 512             # edges per block
SUB = 4               # 128-edge sub-tiles per block
P = 128
KTP = 1344            # 1024 + 256 + 64 contraction size
KPAD = 1408           # padded to 11 chunks of 128
NCHUNK = 11
RSQRT_MAGIC = 0x5F3759DF
NPAIR = N_NODES // 2  # 25000
XR = 128              # bf16 units per node row (120 nodes + 6 pos-halves + 2 pad)

E_CORE = N_EDGES // NCORES                      # 50000
ECLS = 13312                                    # padded edges per parity class
NBLK_CLS = ECLS // BLK                          # 26
NBLK = 4 * NBLK_CLS                             # 104
E_PAD = NBLK * BLK                              # 53248
GROUP = 13                                      # blocks per act-table phase group
GS = GROUP * SUB                                # 32 sub-tiles per group

# cos(pi/2 * sqrt(t)) Taylor coefficients, t = min(d^2/49, 1)
ENV_A = (
    1.0,
    -1.2337005500358182,
    0.25366950654487275,
    -0.020863473217859734,
    0.0009192394784838294,
    -2.5171984603292395e-05,
    4.492184960014096e-07,
)

_compiled = {}


def _patch_walrus_dge_levels():
    """This walrus build compiles with DynamicDMA disabled by default, which
    makes dynamic-offset DMAs crash the exec unit. Append the full
    --dge-levels set to every walrus invocation."""
    import concourse.bass_utils as _bu

    if getattr(_bu, "_dge_patched", False):
        return
    orig = _bu.run_command

    def patched(argv, **kw):
        if argv and "walrus_driver" in str(argv[0]) and not any(
            "dge-levels" in str(a) for a in argv
        ):
            argv = list(argv) + [
                "--dge-levels=io,spill_reload,scalar_dynamic_offset,"
                "vector_dynamic_offsets,dynamic_size,dst_reduce,transpose"
            ]
        return orig(argv, **kw)

    _bu.run_command = patched
    _bu._dge_patched = True


_patch_walrus_dge_levels()


def _patch_drain_and_barrier():
    """The final Tile drain runs on the SP engine, whose Drain lowering in this
    walrus build has no free sync-wait slots (its HWDGE queue waits fill them).
    Hoist the tile-clock waits onto dedicated nop instructions emitted just
    before the drain, one wait per nop."""
    if getattr(tile.TileContext, "_dab_patched", False):
        return

    def patched(self, tick_clock, wait_clock):
        nc = self.nc
        nops = [nc.sync.nop() for _ in range(32)]
        drain_inst = nc.sync.drain()
        from concourse.tile import ScopedClock

        wait_clock.add_sem_waits(
            drain_inst.ins, ScopedClock({None: tick_clock.global_clock})
        )
        si = drain_inst.ins.sync_info
        waits = list(si.on_wait) if si and si.on_wait else []
        if waits:
            assert len(waits) <= len(nops), f"{len(waits)} waits > nop slots"
            si.on_wait = []
            for w, n in zip(waits, nops):
                n.ins.sync_info = mybir.SyncInfo(on_wait=[w], on_update=[])

        nc.all_engine_barrier()
        assert self.sems is not None
        popped = nc._tile_sem_poison_stack.pop()
        assert popped is self._sem_poison
        nc.clear_and_free_semaphores(list(self.sems.allocated().values()))
        nc.all_engine_barrier()

    tile.TileContext._drain_and_barrier = patched
    tile.TileContext._dab_patched = True


_patch_drain_and_barrier()


def _newton_rsqrt(nc, pool, u, n, magic_t, tag, iters=3):
    """rsqrt(u) for u[:, :n] > 0 on the VectorEngine (no ScalarE table)."""
    bits = pool.tile([P, n], I32, tag=f"{tag}_b")
    nc.vector.tensor_copy(out=bits[:].bitcast(F32), in_=u)  # raw bit copy
    nc.vector.tensor_scalar(
        out=bits[:], in0=bits[:], scalar1=1, scalar2=None,
        op0=OP.arith_shift_right,
    )
    yb = pool.tile([P, n], I32, tag=f"{tag}_y")
    nc.vector.tensor_tensor(
        out=yb[:], in0=magic_t[:, 0:1].to_broadcast([P, n]), in1=bits[:],
        op=OP.subtract,
    )
    y = yb[:].bitcast(F32)
    t1 = pool.tile([P, n], F32, tag=f"{tag}_t1")
    for _ in range(iters):
        nc.vector.tensor_mul(t1[:], y, y)
        nc.vector.tensor_mul(t1[:], t1[:], u)
        nc.vector.tensor_scalar(
            out=t1[:], in0=t1[:], scalar1=-0.5, scalar2=1.5, op0=OP.mult, op1=OP.add,
        )
        nc.vector.tensor_mul(y, y, t1[:])
    return yb


def _build(nblocks: int, zero_bias: bool):
    nc = bacc.Bacc("TRN2", target_bir_lowering=False, debug=False)

    nodes_pair = nc.dram_tensor("nodes_pair", (NPAIR, 2 * XR), BF16, kind="ExternalInput").ap()
    xw16 = nc.dram_tensor("xw16", (nblocks, P, 64), I16, kind="ExternalInput").ap()
    geo12 = nc.dram_tensor("geo12", (nblocks * BLK, 12), F32, kind="ExternalInput").ap()
    wflat = nc.dram_tensor("wflat", (KPAD, NS), BF16, kind="ExternalInput").ap()
    dfw1 = nc.dram_tensor("dfw1", (NB, 128), BF16, kind="ExternalInput").ap()
    dfb1 = nc.dram_tensor("dfb1", (1, 128), BF16, kind="ExternalInput").ap()
    dfw2gb = nc.dram_tensor("dfw2gb", (128, 256), BF16, kind="ExternalInput").ap()
    dfb2gb = nc.dram_tensor("dfb2gb", (1, 256), BF16, kind="ExternalInput").ap()
    mlpw1 = nc.dram_tensor("mlpw1", (128, 512), BF16, kind="ExternalInput").ap()
    mlpb1 = nc.dram_tensor("mlpb1", (1, 512), BF16, kind="ExternalInput").ap()
    w2row = nc.dram_tensor("w2row", (1, 512), BF16, kind="ExternalInput").ap()
    b2sc = nc.dram_tensor("b2sc", (1, 1), F32, kind="ExternalInput").ap()
    offs = nc.dram_tensor("offs", (1, NB), F32, kind="ExternalInput").ap()
    out = nc.dram_tensor("out", (nblocks * BLK,), F32, kind="ExternalOutput").ap()

    width = CUTOFF / (NB - 1)
    coeff = 0.5 / (width * width)
    sqc = float(np.sqrt(coeff))

    XGBUFS = GROUP + 6

    with tile.TileContext(nc) as tc:
        with (
            tc.tile_pool(name="const", bufs=1) as constp,
            tc.tile_pool(name="xgp", bufs=XGBUFS) as xgp,
            tc.tile_pool(name="grp", bufs=3) as grpp,
            tc.tile_pool(name="rbp", bufs=2) as rbp,
            tc.tile_pool(name="io", bufs=6) as iop,
            tc.tile_pool(name="geo", bufs=4) as geop,
            tc.tile_pool(name="pfeat", bufs=3) as pfp,
            tc.tile_pool(name="trsb", bufs=4) as trsbp,
            tc.tile_pool(name="work", bufs=4) as workp,
            tc.tile_pool(name="acc", bufs=3) as accp,
            tc.tile_pool(name="ps_t8", bufs=2, space="PSUM") as ps_t8,
            tc.tile_pool(name="ps_mm", bufs=2, space="PSUM") as ps_mm,
            tc.tile_pool(name="ps_hd", bufs=1, space="PSUM") as ps_hd,
            tc.tile_pool(name="ps_g", bufs=2, space="PSUM") as ps_g,
        ):
            # ---- resident constants ----
            identb = constp.tile([P, P], BF16)
            make_identity(nc, identb[:])
            magic_t = constp.tile([P, 1], I32)
            nc.vector.memset(magic_t[:], RSQRT_MAGIC)
            ones_row = constp.tile([1, P], BF16)
            nc.vector.memset(ones_row[:], 1.0)

            w_sb = constp.tile([P, NCHUNK, P], BF16)
            nc.sync.dma_start(out=w_sb[:], in_=wflat.rearrange("(c p) w -> p c w", p=P))
            dfw1_sb = constp.tile([P, 128], BF16)
            nc.sync.dma_start(out=dfw1_sb[0:NB, :], in_=dfw1)
            nc.sync.dma_start(out=dfw1_sb[NB:P, :], in_=dfw1)
            if zero_bias:
                dfw2gb_sb = constp.tile([128, 128], BF16)
                nc.sync.dma_start(out=dfw2gb_sb[:], in_=dfw2gb[:, 0:128])
            else:
                dfw2gb_sb = constp.tile([128, 256], BF16)
                nc.sync.dma_start(out=dfw2gb_sb[:], in_=dfw2gb)
            mlpw1_sb = constp.tile([128, 512], BF16)
            nc.sync.dma_start(out=mlpw1_sb[:], in_=mlpw1)
            w2rep_sb = constp.tile([P, 512], BF16)
            nc.sync.dma_start(out=w2rep_sb[:], in_=w2row.to_broadcast([P, 512]))
            b2_sb = constp.tile([P, 1], F32)
            nc.sync.dma_start(out=b2_sb[:], in_=b2sc.to_broadcast([P, 1]))
            offs_sb = constp.tile([P, NB], F32)
            nc.sync.dma_start(out=offs_sb[:], in_=offs.to_broadcast([P, NB]))
            if not zero_bias:
                dfb1_sb = constp.tile([1, 128], BF16)
                nc.sync.dma_start(out=dfb1_sb[:], in_=dfb1)
                dfb2gb_sb = constp.tile([1, 256], BF16)
                nc.sync.dma_start(out=dfb2gb_sb[:], in_=dfb2gb)
                mlpb1_sb = constp.tile([1, 512], BF16)
                nc.sync.dma_start(out=mlpb1_sb[:], in_=mlpb1)

            xg_tiles = {}

            groups = [range(g, min(g + GROUP, nblocks)) for g in range(0, nblocks, GROUP)]
            for blocks in groups:
                gn = len(blocks)
                g0 = blocks[0]
                # ======== Phase A: gather + geometry + RBF (exp table) ========
                d2g = grpp.tile([P, GS], F32, tag="d2g")
                geog = grpp.tile([P, GROUP, SUB, 12], F32, tag="geog")
                nc.sync.dma_start(
                    out=geog[:, 0:gn, :, :],
                    in_=geo12[g0 * BLK : (g0 + gn) * BLK, :].rearrange(
                        "(g s p) j -> p g s j", p=P, s=SUB
                    ),
                )
                tvp = grpp.tile([P, GROUP, SUB, 3, 3], F32, tag="tvp")
                nc.vector.tensor_tensor(
                    out=tvp[:, 0:gn],
                    in0=geog[:, 0:gn, :, 0:3].unsqueeze(4).to_broadcast([P, gn, SUB, 3, 3]),
                    in1=geog[:, 0:gn, :, 3:12].rearrange("p g s (i j) -> p g s i j", j=3),
                    op=OP.mult,
                )
                tvg = grpp.tile([P, GROUP, SUB, 3], F32, tag="tvg")
                nc.vector.reduce_sum(
                    out=tvg[:, 0:gn], in_=tvp[:, 0:gn].transpose([0, 1, 2, 4, 3]),
                    axis=mybir.AxisListType.X,
                )
                for i, b in enumerate(blocks):
                    cls = b // NBLK_CLS
                    ps, pd = (cls >> 1) & 1, cls & 1

                    xw = iop.tile([P, 64], I16, tag="xw")
                    nc.sync.dma_start(out=xw[:], in_=xw16[b])
                    xg = xgp.tile([P, 2 * SUB, 2 * XR], BF16, tag="xg")
                    nc.gpsimd.dma_gather(
                        out_ap=xg[:], in_ap=nodes_pair[:, :], idxs_ap=xw[:],
                        num_idxs=2 * BLK, num_idxs_reg=2 * BLK, elem_size=2 * XR,
                    )
                    xg_tiles[b] = xg

                    # fp32 positions bit-packed into the bf16 rows
                    p1 = xg[:, 0:SUB, ps * XR + 120 : ps * XR + 126].bitcast(F32)
                    p2 = xg[:, SUB : 2 * SUB, pd * XR + 120 : pd * XR + 126].bitcast(F32)

                    rv = geop.tile([P, SUB, 3], F32, tag="rv")
                    nc.vector.tensor_sub(rv[:], p2, p1)
                    nc.vector.tensor_add(rv[:], rv[:], tvg[:, i])
                    rv2 = geop.tile([P, SUB, 3], F32, tag="rv2")
                    nc.vector.tensor_mul(rv2[:], rv[:], rv[:])
                    nc.vector.reduce_sum(
                        out=d2g[:, i * SUB : (i + 1) * SUB], in_=rv2[:],
                        axis=mybir.AxisListType.X,
                    )

                ng = gn * SUB
                nc.vector.tensor_scalar(
                    out=d2g[:, 0:ng], in0=d2g[:, 0:ng], scalar1=1e-12, scalar2=None,
                    op0=OP.max,
                )
                ry = _newton_rsqrt(nc, grpp, d2g[:, 0:ng], ng, magic_t, "rsq", iters=2)
                dist = grpp.tile([P, GS], F32, tag="dist")
                nc.vector.tensor_mul(dist[:, 0:ng], d2g[:, 0:ng], ry[:].bitcast(F32))

                # envelope: env = p(t)^2, t = min(d2/49, 1)
                tgeo = grpp.tile([P, GS], F32, tag="tgeo")
                nc.vector.tensor_scalar(
                    out=tgeo[:, 0:ng], in0=d2g[:, 0:ng], scalar1=1.0 / 49.0, scalar2=1.0,
                    op0=OP.mult, op1=OP.min,
                )
                envr = grpp.tile([P, GS], F32, tag="envr")
                nc.vector.tensor_scalar(
                    out=envr[:, 0:ng], in0=tgeo[:, 0:ng], scalar1=ENV_A[6], scalar2=None,
                    op0=OP.mult,
                )
                for k in range(5, 0, -1):
                    nc.vector.scalar_tensor_tensor(
                        out=envr[:, 0:ng], in0=envr[:, 0:ng], scalar=ENV_A[k],
                        in1=tgeo[:, 0:ng], op0=OP.add, op1=OP.mult,
                    )
                env = grpp.tile([P, GS], F32, tag="env")
                nc.vector.tensor_scalar(
                    out=env[:, 0:ng], in0=envr[:, 0:ng], scalar1=ENV_A[0], scalar2=None,
                    op0=OP.add,
                )
                nc.vector.tensor_mul(env[:, 0:ng], env[:, 0:ng], env[:, 0:ng])

                # rbf then demb = rbf * env (one Square + one Exp per group)
                rb = rbp.tile([P, GS, NB], F32, tag="rb")
                nc.vector.tensor_tensor(
                    out=rb[:, 0:ng, :],
                    in0=offs_sb[:].unsqueeze(1).to_broadcast([P, ng, NB]),
                    in1=dist[:, 0:ng].unsqueeze(2).to_broadcast([P, ng, NB]),
                    op=OP.subtract,
                )
                nc.scalar.activation(rb[:, 0:ng, :], rb[:, 0:ng, :], AF.Square, scale=sqc)
                demb = grpp.tile([P, GS, NB], BF16, tag="demb")
                if zero_bias:
                    # env is folded into the dfilter silu scale downstream
                    nc.scalar.activation(demb[:, 0:ng, :], rb[:, 0:ng, :], AF.Exp, scale=-1.0)
                else:
                    nc.scalar.activation(rb[:, 0:ng, :], rb[:, 0:ng, :], AF.Exp, scale=-1.0)
                    nc.vector.tensor_tensor(
                        out=demb[:, 0:ng, :], in0=rb[:, 0:ng, :],
                        in1=env[:, 0:ng].unsqueeze(2).to_broadcast([P, ng, NB]),
                        op=OP.mult,
                    )

                # ======== Phase B: TP + LN + dfilter + MLP (silu table) ========
                for j0 in range(0, gn, 2):
                  pair = list(blocks[j0 : j0 + 2])
                  np_pair = len(pair)
                  sumv = geop.tile([P, 2, SUB], F32, tag="sumv")
                  sumsq = geop.tile([P, 2, SUB], F32, tag="sumsq")
                  psmixes = {}
                  for jj, b in enumerate(pair):
                    i = j0 + jj
                    cls = b // NBLK_CLS
                    ps, pd = (cls >> 1) & 1, cls & 1
                    xg = xg_tiles.pop(b)

                    x1 = xg[:, 0:SUB, ps * XR : ps * XR + 120]
                    x2 = xg[:, SUB : 2 * SUB, pd * XR : pd * XR + 120]

                    psmix = ps_mm.tile([P, SUB, NS], F32, tag="psmix")
                    psmixes[b] = psmix

                    # ---- pass 1: tensor product per sub-tile ----
                    for s in range(SUB):
                        ptb0 = pfp.tile([P, 512], BF16, tag="ptb0")
                        ptb1 = pfp.tile([P, 384], BF16, tag="ptb1")
                        a1 = x1[:, s, 0:L0]
                        a2 = x2[:, s, 0:L0]
                        ptb0h = pfp.tile([P, 512], BF16, tag="ptb0h")
                        nc.vector.tensor_tensor(
                            out=ptb0h[:].rearrange("p (u v) -> p u v", v=L0),
                            in0=a1[:, 0:16].unsqueeze(2).to_broadcast([P, 16, L0]),
                            in1=a2.unsqueeze(1).to_broadcast([P, 16, L0]),
                            op=OP.mult,
                        )
                        nc.vector.tensor_tensor(
                            out=ptb0[:].rearrange("p (u v) -> p u v", v=L0),
                            in0=a1[:, 16:32].unsqueeze(2).to_broadcast([P, 16, L0]),
                            in1=a2.unsqueeze(1).to_broadcast([P, 16, L0]),
                            op=OP.mult,
                        )
                        # 0e transposes + copies + matmuls overlap the rest of
                        # the build: first half depends only on ptb0h
                        ptp8 = ps_t8.tile([P, 8, P], BF16, tag="ptp8")
                        for c in range(4):
                            nc.tensor.transpose(
                                ptp8[:, c, :], ptb0h[:, c * P : (c + 1) * P], identb[:]
                            )
                        pts8 = trsbp.tile([P, 8, P], BF16, tag="pts8")
                        nc.scalar.copy(pts8[:, 0:4, :], ptp8[:, 0:4, :])
                        for c in range(4):
                            nc.tensor.transpose(
                                ptp8[:, 4 + c, :], ptb0[:, c * P : (c + 1) * P], identb[:]
                            )
                        nc.scalar.copy(pts8[:, 4:8, :], ptp8[:, 4:8, :])
                        for c in range(8):
                            nc.tensor.matmul(
                                psmix[:, s, :], lhsT=pts8[:, c, :], rhs=w_sb[:, c, :],
                                start=(c == 0), stop=False,
                            )
                        # 1o path: m-major product tile, reduce via 2 contiguous adds
                        b1 = x1[:, s, 32:80].rearrange("p (u m) -> p m u", m=3)
                        b2 = x2[:, s, 32:80].rearrange("p (v m) -> p m v", m=3)
                        pb = workp.tile([P, 3, L1, L1], BF16, tag="pb")
                        nc.vector.tensor_tensor(
                            out=pb[:],
                            in0=b1.unsqueeze(3).to_broadcast([P, 3, L1, L1]),
                            in1=b2.unsqueeze(2).to_broadcast([P, 3, L1, L1]),
                            op=OP.mult,
                        )
                        pbf = pb[:].rearrange("p m u v -> p m (u v)")
                        with nc.allow_low_precision(reason="3-term bf16 add"):
                            t01 = workp.tile([P, L1 * L1], BF16, tag="t01")
                            nc.vector.tensor_add(t01[:], pbf[:, 0, :], pbf[:, 1, :])
                            nc.vector.tensor_add(ptb1[:, 0:256], t01[:], pbf[:, 2, :])
                        # 2e path
                        c1 = x1[:, s, 80:120].rearrange("p (u m) -> p m u", m=5)
                        c2 = x2[:, s, 80:120].rearrange("p (v m) -> p m v", m=5)
                        pc = workp.tile([P, 5, L2, L2], BF16, tag="pc")
                        nc.vector.tensor_tensor(
                            out=pc[:],
                            in0=c1.unsqueeze(3).to_broadcast([P, 5, L2, L2]),
                            in1=c2.unsqueeze(2).to_broadcast([P, 5, L2, L2]),
                            op=OP.mult,
                        )
                        pcf = pc[:].rearrange("p m u v -> p m (u v)")
                        with nc.allow_low_precision(reason="5-term bf16 add"):
                            u01 = workp.tile([P, L2 * L2], BF16, tag="u01")
                            nc.vector.tensor_add(u01[:], pcf[:, 0, :], pcf[:, 1, :])
                            u23 = workp.tile([P, L2 * L2], BF16, tag="u23")
                            nc.vector.tensor_add(u23[:], pcf[:, 2, :], pcf[:, 3, :])
                            nc.vector.tensor_add(u01[:], u01[:], u23[:])
                            nc.vector.tensor_add(ptb1[:, 256:320], u01[:], pcf[:, 4, :])

                        ptp4 = ps_t8.tile([P, 8, P], BF16, tag="ptp8")
                        nc.tensor.transpose(ptp4[:, 0, :], ptb1[:, 0:128], identb[:])
                        nc.tensor.transpose(ptp4[:, 1, :], ptb1[:, 128:256], identb[:])
                        nc.tensor.transpose(ptp4[0:64, 2, :], ptb1[:, 256:320], identb[:])
                        pts4 = trsbp.tile([P, 4, P], BF16, tag="pts4")
                        nc.scalar.copy(pts4[:, 0:3, :], ptp4[:, 0:3, :])
                        nc.tensor.matmul(
                            psmix[:, s, :], lhsT=pts4[:, 0, :], rhs=w_sb[:, 8, :],
                            start=False, stop=False,
                        )
                        nc.tensor.matmul(
                            psmix[:, s, :], lhsT=pts4[:, 1, :], rhs=w_sb[:, 9, :],
                            start=False, stop=False,
                        )
                        nc.tensor.matmul(
                            psmix[:, s, :], lhsT=pts4[0:64, 2, :], rhs=w_sb[0:64, 10, :],
                            start=False, stop=True,
                        )

                    # per-block LN stat reduces into the pair tile
                    nc.vector.reduce_sum(
                        out=sumv[:, jj, :], in_=psmix[:], axis=mybir.AxisListType.X,
                    )
                    sq = workp.tile([P, SUB, NS], BF16, tag="sq")
                    nc.scalar.activation(sq[:], psmix[:], AF.Square)
                    with nc.allow_low_precision(reason="bf16 sumsq reduce"):
                        nc.vector.reduce_sum(
                            out=sumsq[:, jj, :], in_=sq[:], axis=mybir.AxisListType.X,
                        )

                  # ---- pair-level LN scalar chain ----
                  nsx = np_pair * SUB
                  sumvf = sumv[:].rearrange("p j s -> p (j s)")
                  sumsqf = sumsq[:].rearrange("p j s -> p (j s)")
                  muv = geop.tile([P, 2 * SUB], F32, tag="muv")
                  nc.vector.tensor_scalar(
                      out=muv[:, 0:nsx], in0=sumvf[:, 0:nsx], scalar1=1.0 / NS,
                      scalar2=None, op0=OP.mult,
                  )
                  varv = geop.tile([P, 2 * SUB], F32, tag="varv")
                  nc.vector.tensor_mul(varv[:, 0:nsx], muv[:, 0:nsx], muv[:, 0:nsx])
                  nc.vector.scalar_tensor_tensor(
                      out=varv[:, 0:nsx], in0=sumsqf[:, 0:nsx], scalar=1.0 / NS,
                      in1=varv[:, 0:nsx], op0=OP.mult, op1=OP.subtract,
                  )
                  nc.vector.tensor_scalar(
                      out=varv[:, 0:nsx], in0=varv[:, 0:nsx], scalar1=1e-5,
                      scalar2=None, op0=OP.add,
                  )
                  ryl = _newton_rsqrt(nc, geop, varv[:, 0:nsx], nsx, magic_t, "lnr", iters=2)
                  rstd_all = ryl[:].bitcast(F32)
                  tb_all = geop.tile([P, 2 * SUB], F32, tag="tb")
                  nc.vector.scalar_tensor_tensor(
                      out=tb_all[:, 0:nsx], in0=muv[:, 0:nsx], scalar=-1.0,
                      in1=rstd_all, op0=OP.mult, op1=OP.mult,
                  )

                  for jj, b in enumerate(pair):
                    i = j0 + jj
                    cls = b // NBLK_CLS
                    ps, pd = (cls >> 1) & 1, cls & 1
                    e0 = b * BLK
                    sl = slice(e0, e0 + BLK)
                    psmix = psmixes[b]
                    rstd = rstd_all[:, jj * SUB : (jj + 1) * SUB]
                    tb = tb_all[:, jj * SUB : (jj + 1) * SUB]

                    acc = accp.tile([P, SUB], F32, tag="acc")

                    # batched dT transposes: two sub-tiles of demb per transpose
                    dTs = []
                    for h in range(2):
                        dT_ps = ps_t8.tile([P, 8, P], BF16, tag="ptp8")
                        nc.tensor.transpose(
                            dT_ps[:, 0, :],
                            demb[:, (i * SUB + 2 * h) : (i * SUB + 2 * h + 2), :]
                            .rearrange("p s k -> p (s k)"),
                            identb[:],
                        )
                        dT = trsbp.tile([P, P], BF16, tag=f"dT{h}")
                        nc.scalar.copy(dT[:], dT_ps[:, 0, :])
                        dTs.append(dT)

                    # ---- LN apply for all sub-tiles up front (frees psmix) ----
                    ynorms = []
                    for s in range(SUB):
                        ynorm = workp.tile([P, NS], BF16, tag="ynorm")
                        nc.scalar.activation(
                            ynorm[:], psmix[:, s, :], AF.Identity,
                            bias=tb[:, s : s + 1], scale=rstd[:, s : s + 1],
                        )
                        ynorms.append(ynorm)

                    # ---- pass 2: dfilter + final MLP ----
                    for s in range(SUB):
                        ynorm = ynorms[s]
                        h0 = (s % 2) * NB
                        dT = dTs[s // 2][h0 : h0 + NB, :]
                        rhs1 = dfw1_sb[h0 : h0 + NB, :]
                        ph = ps_hd.tile([P, 128], F32, tag="ph")
                        if zero_bias:
                            nc.tensor.matmul(ph[:], lhsT=dT, rhs=rhs1, start=True, stop=True)
                        else:
                            nc.tensor.matmul(ph[:], lhsT=dT, rhs=rhs1, start=True, stop=False)
                            nc.tensor.matmul(ph[:], lhsT=ones_row[:], rhs=dfb1_sb[:], start=False, stop=True)
                        sact = workp.tile([P, 128], BF16, tag="sact")
                        if zero_bias:
                            nc.scalar.activation(
                                sact[:], ph[:], AF.Silu,
                                scale=env[:, i * SUB + s : i * SUB + s + 1],
                            )
                        else:
                            nc.scalar.activation(sact[:], ph[:], AF.Silu)
                        sT_ps = ps_t8.tile([P, 8, P], BF16, tag="ptp8")
                        nc.tensor.transpose(sT_ps[:, 0, :], sact[:], identb[:])
                        sT = trsbp.tile([P, P], BF16, tag="sT")
                        nc.vector.tensor_copy(sT[:], sT_ps[:, 0, :])
                        rg = workp.tile([P, 128], BF16, tag="rg")
                        if zero_bias:
                            pdf = ps_hd.tile([P, 128], F32, tag="pdf")
                            nc.tensor.matmul(pdf[:], lhsT=sT[:], rhs=dfw2gb_sb[:], start=True, stop=True)
                            nc.vector.tensor_mul(rg[:], ynorm[:], pdf[:])
                        else:
                            pdf = ps_hd.tile([P, 256], F32, tag="pdf")
                            nc.tensor.matmul(pdf[:], lhsT=sT[:], rhs=dfw2gb_sb[:], start=True, stop=False)
                            nc.tensor.matmul(pdf[:], lhsT=ones_row[:], rhs=dfb2gb_sb[:], start=False, stop=True)
                            nc.vector.tensor_mul(rg[:], ynorm[:], pdf[:, 0:128])
                            nc.vector.tensor_add(rg[:], rg[:], pdf[:, 128:256])

                        rT_ps = ps_t8.tile([P, 8, P], BF16, tag="ptp8")
                        nc.tensor.transpose(rT_ps[:, 0, :], rg[:], identb[:])
                        rT = trsbp.tile([P, P], BF16, tag="rT")
                        nc.scalar.copy(rT[:], rT_ps[:, 0, :])
                        pg2 = ps_g.tile([P, 512], F32, tag="pg")
                        if zero_bias:
                            nc.tensor.matmul(pg2[:], lhsT=rT[:], rhs=mlpw1_sb[:], start=True, stop=True)
                        else:
                            nc.tensor.matmul(pg2[:], lhsT=rT[:], rhs=mlpw1_sb[:], start=True, stop=False)
                            nc.tensor.matmul(pg2[:], lhsT=ones_row[:], rhs=mlpb1_sb[:], start=False, stop=True)
                        gact = workp.tile([P, 512], BF16, tag="gact")
                        nc.scalar.activation(gact[:], pg2[:], AF.Silu)
                        scr = workp.tile([P, 512], BF16, tag="scr")
                        nc.vector.scalar_tensor_tensor(
                            out=scr[:], in0=gact[:], scalar=1.0, in1=w2rep_sb[:],
                            op0=OP.mult, op1=OP.mult,
                            accum_out=acc[:, s : s + 1],
                        )

                    if not zero_bias:
                        nc.vector.tensor_scalar(
                            out=acc[:], in0=acc[:], scalar1=b2_sb[:, 0:1], scalar2=None,
                            op0=OP.add,
                        )
                    nc.sync.dma_start(out=out[sl].rearrange("(s p) -> p s", p=P), in_=acc[:])

    nc.compile()
    return nc


def _get_compiled(zero_bias: bool = True):
    if zero_bias not in _compiled:
        _compiled[zero_bias] = _build(NBLK, zero_bias)
    return _compiled[zero_bias]


def _wrap16(idx_block):
    """int array [512] -> dma_gather wrapped int16 layout [128, 32]
    (index j at [j%16, j//16], replicated across the 8 gpsimd cores)."""
    w = idx_block.astype(np.int16).reshape(-1, 16).T  # [16, n/16]
    return np.tile(w, (8, 1))


def _prep(inputs):
    nodes = np.asarray(inputs["nodes"], np.float32)
    edge_index = np.asarray(inputs["edge_index"]).astype(np.int64)
    graph_batch = np.asarray(inputs["graph_batch"]).astype(np.int64)
    cell = np.asarray(inputs["cell"], np.float32).reshape(32, 9)
    edge_shift = np.asarray(inputs["edge_shift"], np.float32)
    pos = np.asarray(inputs["pos"], np.float32)

    # bf16 pair-row node table with fp32 pos bit-packed at units 120:126
    row_u16 = np.zeros((N_NODES, XR), np.uint16)
    row_u16[:, 0:NODE_DIM] = nodes.astype(ml_dtypes.bfloat16).view(np.uint16)
    row_u16[:, 120:126] = pos.view(np.uint16).reshape(N_NODES, 6)
    nodes_pair = row_u16.reshape(NPAIR, 2 * XR).view(ml_dtypes.bfloat16)

    alpha = 1.0 / np.sqrt(float(L0 * L0 + L1 * L1 + L2 * L2))
    w0 = np.asarray(inputs["W0"], np.float32).reshape(L0 * L0, NS) * alpha
    w1 = np.asarray(inputs["W1"], np.float32).reshape(L1 * L1, NS) * (alpha / np.sqrt(3.0))
    w2 = np.asarray(inputs["W2"], np.float32).reshape(L2 * L2, NS) * (alpha / np.sqrt(5.0))
    wflat = np.zeros((KPAD, NS), np.float32)
    wflat[0:1024] = w0
    wflat[1024:1280] = w1
    wflat[1280:1344] = w2

    ln_g = np.asarray(inputs["ln_g"], np.float32)
    ln_b = np.asarray(inputs["ln_b"], np.float32)
    df_w2 = np.asarray(inputs["df_w2"], np.float32)
    df_b2 = np.asarray(inputs["df_b2"], np.float32)
    dfw2gb = np.concatenate([df_w2 * ln_g[None, :], df_w2 * ln_b[None, :]], axis=1)
    dfb2gb = np.concatenate([df_b2 * ln_g, df_b2 * ln_b])[None, :]

    zero_bias = (
        not np.any(np.asarray(inputs["df_b1"]))
        and not np.any(np.asarray(inputs["df_b2"]))
        and not np.any(np.asarray(inputs["mlp_b1"]))
        and not np.any(np.asarray(inputs["mlp_b2"]))
        and not np.any(ln_b)
    )

    bf = lambda a: np.ascontiguousarray(a).astype(ml_dtypes.bfloat16)

    common = {
        "nodes_pair": nodes_pair,
        "wflat": bf(wflat),
        "dfw1": bf(np.asarray(inputs["df_w1"], np.float32)),
        "dfb1": bf(np.asarray(inputs["df_b1"], np.float32)[None, :]),
        "dfw2gb": bf(dfw2gb),
        "dfb2gb": bf(dfb2gb),
        "mlpw1": bf(np.asarray(inputs["mlp_w1"], np.float32)),
        "mlpb1": bf(np.asarray(inputs["mlp_b1"], np.float32)[None, :]),
        "w2row": bf(np.asarray(inputs["mlp_w2"], np.float32).T),
        "b2sc": np.asarray(inputs["mlp_b2"], np.float32).reshape(1, 1),
        "offs": np.linspace(0.0, CUTOFF, NB, dtype=np.float32)[None, :],
    }

    in_maps = []
    outmaps = []
    for c in range(NCORES):
        lo, hi = c * E_CORE, (c + 1) * E_CORE
        src = edge_index[0, lo:hi]
        dst = edge_index[1, lo:hi]
        esh = edge_shift[lo:hi]
        key = ((src & 1) << 1) | (dst & 1)

        srcp = np.zeros(E_PAD, np.int64)
        dstp = np.zeros(E_PAD, np.int64)
        geo = np.zeros((E_PAD, 12), np.float32)
        outmap = np.full(E_PAD, -1, np.int64)
        for cls in range(4):
            idxs = np.nonzero(key == cls)[0]
            n = len(idxs)
            assert n <= ECLS, f"class {cls} overflow: {n} > {ECLS}"
            base = cls * ECLS
            srcp[base : base + n] = src[idxs]
            dstp[base : base + n] = dst[idxs]
            geo[base : base + n, 0:3] = esh[idxs]
            geo[base : base + n, 3:12] = cell[graph_batch[src[idxs]]]
            outmap[base : base + n] = idxs

        xw = np.zeros((NBLK, P, 64), np.int16)
        for b in range(NBLK):
            sb = srcp[b * BLK : (b + 1) * BLK]
            db = dstp[b * BLK : (b + 1) * BLK]
            xw[b, :, 0:32] = _wrap16(sb >> 1)
            xw[b, :, 32:64] = _wrap16(db >> 1)

        m = dict(common)
        m["xw16"] = xw
        m["geo12"] = geo
        in_maps.append(m)
        outmaps.append(outmap)
    return in_maps, outmaps, zero_bias


def _gather_out(res, outmaps, b2_host=0.0):
    full = np.empty((N_EDGES,), np.float32)
    for c in range(NCORES):
        dev = np.asarray(res.results[c]["out"])
        outmap = outmaps[c]
        valid = outmap >= 0
        full[c * E_CORE + outmap[valid]] = dev[valid]
    return full.reshape(N_EDGES, 1)


def kernel(**inputs) -> np.ndarray:
    in_maps, outmaps, zero_bias = _prep(inputs)
    nc = _get_compiled(zero_bias)
    res = run_bass_kernel_spmd(nc, in_maps, core_ids=list(range(NCORES)))
    return _gather_out(res, outmaps)


# revision 37
# speedup vs baseline: 1.0233x; 1.0233x over previous
"""Trainium2 Bass kernel for nn_ExchangeBlock (GNN message passing / e3nn-style
tensor-product edge block), SPMD across 8 NeuronCores.

Sharding: edges across the 8 cores; node features and params replicated.

v4 design notes:
- Edges are host-sorted into 4 parity classes (src&1, dst&1) so the pair-row
  parity select becomes a compile-time AP slice: no masks, no predicated
  copies, no gpsimd copy traffic at all.
- ONE dma_gather per block: the node table is bf16 pair rows (512B) with the
  fp32 position bit-packed into units 120:126 of each row, so geometry and
  the tensor product share the same gathered tile.  cell[graph_batch[src]]
  is a host-side index prep (like the baseline's graph_batch[src]) and is
  streamed per edge together with edge_shift.
- Blocks run in groups of 8 with two ScalarE activation-table phases per
  group (exp set: RBF; silu set: MLPs).  The per-edge scalar chain (Newton
  rsqrt, cutoff envelope as an exact degree-6 polynomial in d^2, RBF, demb)
  is batched once per group, so phase A is a handful of wide ops and the
  Square/Exp are single instructions the OoO scheduler cannot shred.
- The TP runs as outer-product features P[e,1344] built on DVE (bf16), PE
  transposes of P chunks (one full PSUM bank per 8 chunks), and accumulated
  128x128 matmuls against pre-scaled flattened weights.  The 1o/2e paths
  build m-major product tiles and reduce with 2 contiguous bf16 adds
  instead of a slow innermost-3 reduce.
- LN stats via one PSUM reduce + a batched ScalarE Square + one bf16
  reduce; the final w2 contraction fuses multiply+reduce into one
  scalar_tensor_tensor with accum_out.
- All MLP biases in this problem are exactly zero; _prep detects that and
  compiles the bias-free variant (rank-1 PE bias matmuls otherwise).
"""

import sys

sys.path.insert(0, "/opt/trn_rl_repo")

import numpy as np
import ml_dtypes

import concourse.bass as bass
import concourse.mybir as mybir
import concourse.tile as tile
from concourse import bacc
from concourse.bass_utils import run_bass_kernel_spmd
from concourse.masks import make_identity

F32 = mybir.dt.float32
BF16 = mybir.dt.bfloat16
I32 = mybir.dt.int32
I16 = mybir.dt.int16
AF = mybir.ActivationFunctionType
OP = mybir.AluOpType

# Problem constants
L0, L1, L2 = 32, 16, 8
NS = 128
NB = 64
CUTOFF = 7.0
N_NODES = 50000
N_EDGES = 400000
NODE_DIM = 120
NCORES = 8

BLK = 512             # edges per block
SUB = 4               # 128-edge sub-tiles per block
P = 128
KTP = 1344            # 1024 + 256 + 64 contraction size
KPAD = 1408           # padded to 11 chunks of 128
NCHUNK = 11
RSQRT_MAGIC = 0x5F3759DF
NPAIR = N_NODES // 2  # 25000
XR = 128              # bf16 units per node row (120 nodes + 6 pos-halves + 2 pad)

E_CORE = N_EDGES // NCORES                      # 50000
ECLS = 13312                                    # padded edges per parity class
NBLK_CLS = ECLS // BLK                          # 26
NBLK = 4 * NBLK_CLS                             # 104
E_PAD = NBLK * BLK                              # 53248
GROUP = 13                                      # blocks per act-table phase group
GS = GROUP * SUB                                # 32 sub-tiles per group

# cos(pi/2 * sqrt(t)) Taylor coefficients, t = min(d^2/49, 1)
ENV_A = (
    1.0,
    -1.2337005500358182,
    0.25366950654487275,
    -0.020863473217859734,
    0.0009192394784838294,
    -2.5171984603292395e-05,
    4.492184960014096e-07,
)

_compiled = {}


def _patch_walrus_dge_levels():
    """This walrus build compiles with DynamicDMA disabled by default, which
    makes dynamic-offset DMAs crash the exec unit. Append the full
    --dge-levels set to every walrus invocation."""
    import concourse.bass_utils as _bu

    if getattr(_bu, "_dge_patched", False):
        return
    orig = _bu.run_command

    def patched(argv, **kw):
        if argv and "walrus_driver" in str(argv[0]) and not any(
            "dge-levels" in str(a) for a in argv
        ):
            argv = list(argv) + [
                "--dge-levels=io,spill_reload,scalar_dynamic_offset,"
                "vector_dynamic_offsets,dynamic_size,dst_reduce,transpose"
            ]
        return orig(argv, **kw)

    _bu.run_command = patched
    _bu._dge_patched = True


_patch_walrus_dge_levels()


def _patch_drain_and_barrier():
    """The final Tile drain runs on the SP engine, whose Drain lowering in this
    walrus build has no free sync-wait slots (its HWDGE queue waits fill them).
    Hoist the tile-clock waits onto dedicated nop instructions emitted just
    before the drain, one wait per nop."""
    if getattr(tile.TileContext, "_dab_patched", False):
        return

    def patched(self, tick_clock, wait_clock):
        nc = self.nc
        nops = [nc.sync.nop() for _ in range(32)]
        drain_inst = nc.sync.drain()
        from concourse.tile import ScopedClock

        wait_clock.add_sem_waits(
            drain_inst.ins, ScopedClock({None: tick_clock.global_clock})
        )
        si = drain_inst.ins.sync_info
        waits = list(si.on_wait) if si and si.on_wait else []
        if waits:
            assert len(waits) <= len(nops), f"{len(waits)} waits > nop slots"
            si.on_wait = []
            for w, n in zip(waits, nops):
                n.ins.sync_info = mybir.SyncInfo(on_wait=[w], on_update=[])

        nc.all_engine_barrier()
        assert self.sems is not None
        popped = nc._tile_sem_poison_stack.pop()
        assert popped is self._sem_poison
        nc.clear_and_free_semaphores(list(self.sems.allocated().values()))
        nc.all_engine_barrier()

    tile.TileContext._drain_and_barrier = patched
    tile.TileContext._dab_patched = True


_patch_drain_and_barrier()


def _newton_rsqrt(nc, pool, u, n, magic_t, tag, iters=3):
    """rsqrt(u) for u[:, :n] > 0 on the VectorEngine (no ScalarE table)."""
    bits = pool.tile([P, n], I32, tag=f"{tag}_b")
    nc.vector.tensor_copy(out=bits[:].bitcast(F32), in_=u)  # raw bit copy
    nc.vector.tensor_scalar(
        out=bits[:], in0=bits[:], scalar1=1, scalar2=None,
        op0=OP.arith_shift_right,
    )
    yb = pool.tile([P, n], I32, tag=f"{tag}_y")
    nc.vector.tensor_tensor(
        out=yb[:], in0=magic_t[:, 0:1].to_broadcast([P, n]), in1=bits[:],
        op=OP.subtract,
    )
    y = yb[:].bitcast(F32)
    t1 = pool.tile([P, n], F32, tag=f"{tag}_t1")
    for _ in range(iters):
        nc.vector.tensor_mul(t1[:], y, y)
        nc.vector.tensor_mul(t1[:], t1[:], u)
        nc.vector.tensor_scalar(
            out=t1[:], in0=t1[:], scalar1=-0.5, scalar2=1.5, op0=OP.mult, op1=OP.add,
        )
        nc.vector.tensor_mul(y, y, t1[:])
    return yb


def _build(nblocks: int, zero_bias: bool):
    nc = bacc.Bacc("TRN2", target_bir_lowering=False, debug=False)

    nodes_pair = nc.dram_tensor("nodes_pair", (NPAIR, 2 * XR), BF16, kind="ExternalInput").ap()
    xw16 = nc.dram_tensor("xw16", (nblocks, P, 64), I16, kind="ExternalInput").ap()
    geo12 = nc.dram_tensor("geo12", (nblocks * BLK, 12), F32, kind="ExternalInput").ap()
    wflat = nc.dram_tensor("wflat", (KPAD, NS), BF16, kind="ExternalInput").ap()
    dfw1 = nc.dram_tensor("dfw1", (NB, 128), BF16, kind="ExternalInput").ap()
    dfb1 = nc.dram_tensor("dfb1", (1, 128), BF16, kind="ExternalInput").ap()
    dfw2gb = nc.dram_tensor("dfw2gb", (128, 256), BF16, kind="ExternalInput").ap()
    dfb2gb = nc.dram_tensor("dfb2gb", (1, 256), BF16, kind="ExternalInput").ap()
    mlpw1 = nc.dram_tensor("mlpw1", (128, 512), BF16, kind="ExternalInput").ap()
    mlpb1 = nc.dram_tensor("mlpb1", (1, 512), BF16, kind="ExternalInput").ap()
    w2row = nc.dram_tensor("w2row", (1, 512), BF16, kind="ExternalInput").ap()
    b2sc = nc.dram_tensor("b2sc", (1, 1), F32, kind="ExternalInput").ap()
    offs = nc.dram_tensor("offs", (1, NB), F32, kind="ExternalInput").ap()
    out = nc.dram_tensor("out", (nblocks * BLK,), F32, kind="ExternalOutput").ap()

    width = CUTOFF / (NB - 1)
    coeff = 0.5 / (width * width)
    sqc = float(np.sqrt(coeff))

    XGBUFS = GROUP + 6

    with tile.TileContext(nc) as tc:
        with (
            tc.tile_pool(name="const", bufs=1) as constp,
            tc.tile_pool(name="xgp", bufs=XGBUFS) as xgp,
            tc.tile_pool(name="grp", bufs=3) as grpp,
            tc.tile_pool(name="rbp", bufs=2) as rbp,
            tc.tile_pool(name="io", bufs=6) as iop,
            tc.tile_pool(name="geo", bufs=4) as geop,
            tc.tile_pool(name="pfeat", bufs=3) as pfp,
            tc.tile_pool(name="trsb", bufs=4) as trsbp,
            tc.tile_pool(name="work", bufs=4) as workp,
            tc.tile_pool(name="acc", bufs=3) as accp,
            tc.tile_pool(name="ps_t8", bufs=2, space="PSUM") as ps_t8,
            tc.tile_pool(name="ps_mm", bufs=2, space="PSUM") as ps_mm,
            tc.tile_pool(name="ps_hd", bufs=1, space="PSUM") as ps_hd,
            tc.tile_pool(name="ps_g", bufs=2, space="PSUM") as ps_g,
        ):
            # ---- resident constants ----
            identb = constp.tile([P, P], BF16)
            make_identity(nc, identb[:])
            magic_t = constp.tile([P, 1], I32)
            nc.vector.memset(magic_t[:], RSQRT_MAGIC)
            ones_row = constp.tile([1, P], BF16)
            nc.vector.memset(ones_row[:], 1.0)

            w_sb = constp.tile([P, NCHUNK, P], BF16)
            nc.sync.dma_start(out=w_sb[:], in_=wflat.rearrange("(c p) w -> p c w", p=P))
            dfw1_sb = constp.tile([P, 128], BF16)
            nc.sync.dma_start(out=dfw1_sb[0:NB, :], in_=dfw1)
            nc.sync.dma_start(out=dfw1_sb[NB:P, :], in_=dfw1)
            if zero_bias:
                dfw2gb_sb = constp.tile([128, 128], BF16)
                nc.sync.dma_start(out=dfw2gb_sb[:], in_=dfw2gb[:, 0:128])
            else:
                dfw2gb_sb = constp.tile([128, 256], BF16)
                nc.sync.dma_start(out=dfw2gb_sb[:], in_=dfw2gb)
            mlpw1_sb = constp.tile([128, 512], BF16)
            nc.sync.dma_start(out=mlpw1_sb[:], in_=mlpw1)
            w2rep_sb = constp.tile([P, 512], BF16)
            nc.sync.dma_start(out=w2rep_sb[:], in_=w2row.to_broadcast([P, 512]))
            b2_sb = constp.tile([P, 1], F32)
            nc.sync.dma_start(out=b2_sb[:], in_=b2sc.to_broadcast([P, 1]))
            offs_sb = constp.tile([P, NB], F32)
            nc.sync.dma_start(out=offs_sb[:], in_=offs.to_broadcast([P, NB]))
            if not zero_bias:
                dfb1_sb = constp.tile([1, 128], BF16)
                nc.sync.dma_start(out=dfb1_sb[:], in_=dfb1)
                dfb2gb_sb = constp.tile([1, 256], BF16)
                nc.sync.dma_start(out=dfb2gb_sb[:], in_=dfb2gb)
                mlpb1_sb = constp.tile([1, 512], BF16)
                nc.sync.dma_start(out=mlpb1_sb[:], in_=mlpb1)

            xg_tiles = {}

            groups = [range(g, min(g + GROUP, nblocks)) for g in range(0, nblocks, GROUP)]
            for blocks in groups:
                gn = len(blocks)
                g0 = blocks[0]
                # ======== Phase A: gather + geometry + RBF (exp table) ========
                d2g = grpp.tile([P, GS], F32, tag="d2g")
                geog = grpp.tile([P, GROUP, SUB, 12], F32, tag="geog")
                nc.sync.dma_start(
                    out=geog[:, 0:gn, :, :],
                    in_=geo12[g0 * BLK : (g0 + gn) * BLK, :].rearrange(
                        "(g s p) j -> p g s j", p=P, s=SUB
                    ),
                )
                tvp = grpp.tile([P, GROUP, SUB, 3, 3], F32, tag="tvp")
                nc.vector.tensor_tensor(
                    out=tvp[:, 0:gn],
                    in0=geog[:, 0:gn, :, 0:3].unsqueeze(4).to_broadcast([P, gn, SUB, 3, 3]),
                    in1=geog[:, 0:gn, :, 3:12].rearrange("p g s (i j) -> p g s i j", j=3),
                    op=OP.mult,
                )
                tvg = grpp.tile([P, GROUP, SUB, 3], F32, tag="tvg")
                nc.vector.reduce_sum(
                    out=tvg[:, 0:gn], in_=tvp[:, 0:gn].transpose([0, 1, 2, 4, 3]),
                    axis=mybir.AxisListType.X,
                )
                for i, b in enumerate(blocks):
                    cls = b // NBLK_CLS
                    ps, pd = (cls >> 1) & 1, cls & 1

                    xw = iop.tile([P, 64], I16, tag="xw")
                    nc.sync.dma_start(out=xw[:], in_=xw16[b])
                    xg = xgp.tile([P, 2 * SUB, 2 * XR], BF16, tag="xg")
                    nc.gpsimd.dma_gather(
                        out_ap=xg[:], in_ap=nodes_pair[:, :], idxs_ap=xw[:],
                        num_idxs=2 * BLK, num_idxs_reg=2 * BLK, elem_size=2 * XR,
                    )
                    xg_tiles[b] = xg

                    # fp32 positions bit-packed into the bf16 rows
                    p1 = xg[:, 0:SUB, ps * XR + 120 : ps * XR + 126].bitcast(F32)
                    p2 = xg[:, SUB : 2 * SUB, pd * XR + 120 : pd * XR + 126].bitcast(F32)

                    rv = geop.tile([P, SUB, 3], F32, tag="rv")
                    nc.vector.tensor_sub(rv[:], p2, p1)
                    nc.vector.tensor_add(rv[:], rv[:], tvg[:, i])
                    rv2 = geop.tile([P, SUB, 3], F32, tag="rv2")
                    nc.vector.tensor_mul(rv2[:], rv[:], rv[:])
                    nc.vector.reduce_sum(
                        out=d2g[:, i * SUB : (i + 1) * SUB], in_=rv2[:],
                        axis=mybir.AxisListType.X,
                    )

                ng = gn * SUB
                nc.vector.tensor_scalar(
                    out=d2g[:, 0:ng], in0=d2g[:, 0:ng], scalar1=1e-12, scalar2=None,
                    op0=OP.max,
                )
                ry = _newton_rsqrt(nc, grpp, d2g[:, 0:ng], ng, magic_t, "rsq", iters=2)
                dist = grpp.tile([P, GS], F32, tag="dist")
                nc.vector.tensor_mul(dist[:, 0:ng], d2g[:, 0:ng], ry[:].bitcast(F32))

                # envelope: env = p(t)^2, t = min(d2/49, 1)
                tgeo = grpp.tile([P, GS], F32, tag="tgeo")
                nc.vector.tensor_scalar(
                    out=tgeo[:, 0:ng], in0=d2g[:, 0:ng], scalar1=1.0 / 49.0, scalar2=1.0,
                    op0=OP.mult, op1=OP.min,
                )
                envr = grpp.tile([P, GS], F32, tag="envr")
                nc.vector.tensor_scalar(
                    out=envr[:, 0:ng], in0=tgeo[:, 0:ng], scalar1=ENV_A[6], scalar2=None,
                    op0=OP.mult,
                )
                for k in range(5, 0, -1):
                    nc.vector.scalar_tensor_tensor(
                        out=envr[:, 0:ng], in0=envr[:, 0:ng], scalar=ENV_A[k],
                        in1=tgeo[:, 0:ng], op0=OP.add, op1=OP.mult,
                    )
                env = grpp.tile([P, GS], F32, tag="env")
                nc.vector.tensor_scalar(
                    out=env[:, 0:ng], in0=envr[:, 0:ng], scalar1=ENV_A[0], scalar2=None,
                    op0=OP.add,
                )
                nc.vector.tensor_mul(env[:, 0:ng], env[:, 0:ng], env[:, 0:ng])

                # rbf then demb = rbf * env (one Square + one Exp per group)
                rb = rbp.tile([P, GS, NB], F32, tag="rb")
                nc.vector.tensor_tensor(
                    out=rb[:, 0:ng, :],
                    in0=offs_sb[:].unsqueeze(1).to_broadcast([P, ng, NB]),
                    in1=dist[:, 0:ng].unsqueeze(2).to_broadcast([P, ng, NB]),
                    op=OP.subtract,
                )
                nc.scalar.activation(rb[:, 0:ng, :], rb[:, 0:ng, :], AF.Square, scale=sqc)
                demb = grpp.tile([P, GS, NB], BF16, tag="demb")
                if zero_bias:
                    # env is folded into the dfilter silu scale downstream
                    nc.scalar.activation(demb[:, 0:ng, :], rb[:, 0:ng, :], AF.Exp, scale=-1.0)
                else:
                    nc.scalar.activation(rb[:, 0:ng, :], rb[:, 0:ng, :], AF.Exp, scale=-1.0)
                    nc.vector.tensor_tensor(
                        out=demb[:, 0:ng, :], in0=rb[:, 0:ng, :],
                        in1=env[:, 0:ng].unsqueeze(2).to_broadcast([P, ng, NB]),
                        op=OP.mult,
                    )

                # ======== Phase B: TP + LN + dfilter + MLP (silu table) ========
                for j0 in range(0, gn, 2):
                  pair = list(blocks[j0 : j0 + 2])
                  np_pair = len(pair)
                  sumv = geop.tile([P, 2, SUB], F32, tag="sumv")
                  sumsq = geop.tile([P, 2, SUB], F32, tag="sumsq")
                  psmixes = {}
                  for jj, b in enumerate(pair):
                    i = j0 + jj
                    cls = b // NBLK_CLS
                    ps, pd = (cls >> 1) & 1, cls & 1
                    xg = xg_tiles.pop(b)

                    x1 = xg[:, 0:SUB, ps * XR : ps * XR + 120]
                    x2 = xg[:, SUB : 2 * SUB, pd * XR : pd * XR + 120]

                    psmix = ps_mm.tile([P, SUB, NS], F32, tag="psmix")
                    psmixes[b] = psmix

                    # ---- pass 1: tensor product per sub-tile ----
                    for s in range(SUB):
                        ptb0 = pfp.tile([P, 1024], BF16, tag="ptb0")
                        ptb1 = pfp.tile([P, 384], BF16, tag="ptb1")
                        a1 = x1[:, s, 0:L0]
                        a2 = x2[:, s, 0:L0]
                        nc.vector.tensor_tensor(
                            out=ptb0[:].rearrange("p (u v) -> p u v", v=L0),
                            in0=a1.unsqueeze(2).to_broadcast([P, L0, L0]),
                            in1=a2.unsqueeze(1).to_broadcast([P, L0, L0]),
                            op=OP.mult,
                        )
                        # 0e transposes + copy + matmuls depend only on ptb0,
                        # overlapping the 1o/2e build below
                        ptp8 = ps_t8.tile([P, 8, P], BF16, tag="ptp8")
                        for c in range(8):
                            nc.tensor.transpose(
                                ptp8[:, c, :], ptb0[:, c * P : (c + 1) * P], identb[:]
                            )
                        pts8 = trsbp.tile([P, 8, P], BF16, tag="pts8")
                        nc.scalar.copy(pts8[:], ptp8[:])
                        for c in range(8):
                            nc.tensor.matmul(
                                psmix[:, s, :], lhsT=pts8[:, c, :], rhs=w_sb[:, c, :],
                                start=(c == 0), stop=False,
                            )
                        # 1o path: m-major product tile, reduce via 2 contiguous adds
                        b1 = x1[:, s, 32:80].rearrange("p (u m) -> p m u", m=3)
                        b2 = x2[:, s, 32:80].rearrange("p (v m) -> p m v", m=3)
                        pb = workp.tile([P, 3, L1, L1], BF16, tag="pb")
                        nc.vector.tensor_tensor(
                            out=pb[:],
                            in0=b1.unsqueeze(3).to_broadcast([P, 3, L1, L1]),
                            in1=b2.unsqueeze(2).to_broadcast([P, 3, L1, L1]),
                            op=OP.mult,
                        )
                        pbf = pb[:].rearrange("p m u v -> p m (u v)")
                        with nc.allow_low_precision(reason="3-term bf16 add"):
                            t01 = workp.tile([P, L1 * L1], BF16, tag="t01")
                            nc.vector.tensor_add(t01[:], pbf[:, 0, :], pbf[:, 1, :])
                            nc.vector.tensor_add(ptb1[:, 0:256], t01[:], pbf[:, 2, :])
                        # 2e path
                        c1 = x1[:, s, 80:120].rearrange("p (u m) -> p m u", m=5)
                        c2 = x2[:, s, 80:120].rearrange("p (v m) -> p m v", m=5)
                        pc = workp.tile([P, 5, L2, L2], BF16, tag="pc")
                        nc.vector.tensor_tensor(
                            out=pc[:],
                            in0=c1.unsqueeze(3).to_broadcast([P, 5, L2, L2]),
                            in1=c2.unsqueeze(2).to_broadcast([P, 5, L2, L2]),
                            op=OP.mult,
                        )
                        pcf = pc[:].rearrange("p m u v -> p m (u v)")
                        with nc.allow_low_precision(reason="5-term bf16 add"):
                            u01 = workp.tile([P, L2 * L2], BF16, tag="u01")
                            nc.vector.tensor_add(u01[:], pcf[:, 0, :], pcf[:, 1, :])
                            u23 = workp.tile([P, L2 * L2], BF16, tag="u23")
                            nc.vector.tensor_add(u23[:], pcf[:, 2, :], pcf[:, 3, :])
                            nc.vector.tensor_add(u01[:], u01[:], u23[:])
                            nc.vector.tensor_add(ptb1[:, 256:320], u01[:], pcf[:, 4, :])

                        ptp4 = ps_t8.tile([P, 8, P], BF16, tag="ptp8")
                        nc.tensor.transpose(ptp4[:, 0, :], ptb1[:, 0:128], identb[:])
                        nc.tensor.transpose(ptp4[:, 1, :], ptb1[:, 128:256], identb[:])
                        nc.tensor.transpose(ptp4[0:64, 2, :], ptb1[:, 256:320], identb[:])
                        pts4 = trsbp.tile([P, 4, P], BF16, tag="pts4")
                        nc.scalar.copy(pts4[:, 0:3, :], ptp4[:, 0:3, :])
                        nc.tensor.matmul(
                            psmix[:, s, :], lhsT=pts4[:, 0, :], rhs=w_sb[:, 8, :],
                            start=False, stop=False,
                        )
                        nc.tensor.matmul(
                            psmix[:, s, :], lhsT=pts4[:, 1, :], rhs=w_sb[:, 9, :],
                            start=False, stop=False,
                        )
                        nc.tensor.matmul(
                            psmix[:, s, :], lhsT=pts4[0:64, 2, :], rhs=w_sb[0:64, 10, :],
                            start=False, stop=True,
                        )

                    # per-block LN stat reduces into the pair tile
                    nc.vector.reduce_sum(
                        out=sumv[:, jj, :], in_=psmix[:], axis=mybir.AxisListType.X,
                    )
                    sq = workp.tile([P, SUB, NS], BF16, tag="sq")
                    nc.scalar.activation(sq[:], psmix[:], AF.Square)
                    with nc.allow_low_precision(reason="bf16 sumsq reduce"):
                        nc.vector.reduce_sum(
                            out=sumsq[:, jj, :], in_=sq[:], axis=mybir.AxisListType.X,
                        )

                  # ---- pair-level LN scalar chain ----
                  nsx = np_pair * SUB
                  sumvf = sumv[:].rearrange("p j s -> p (j s)")
                  sumsqf = sumsq[:].rearrange("p j s -> p (j s)")
                  muv = geop.tile([P, 2 * SUB], F32, tag="muv")
                  nc.vector.tensor_scalar(
                      out=muv[:, 0:nsx], in0=sumvf[:, 0:nsx], scalar1=1.0 / NS,
                      scalar2=None, op0=OP.mult,
                  )
                  varv = geop.tile([P, 2 * SUB], F32, tag="varv")
                  nc.vector.tensor_mul(varv[:, 0:nsx], muv[:, 0:nsx], muv[:, 0:nsx])
                  nc.vector.scalar_tensor_tensor(
                      out=varv[:, 0:nsx], in0=sumsqf[:, 0:nsx], scalar=1.0 / NS,
                      in1=varv[:, 0:nsx], op0=OP.mult, op1=OP.subtract,
                  )
                  nc.vector.tensor_scalar(
                      out=varv[:, 0:nsx], in0=varv[:, 0:nsx], scalar1=1e-5,
                      scalar2=None, op0=OP.add,
                  )
                  ryl = _newton_rsqrt(nc, geop, varv[:, 0:nsx], nsx, magic_t, "lnr", iters=2)
                  rstd_all = ryl[:].bitcast(F32)
                  tb_all = geop.tile([P, 2 * SUB], F32, tag="tb")
                  nc.vector.scalar_tensor_tensor(
                      out=tb_all[:, 0:nsx], in0=muv[:, 0:nsx], scalar=-1.0,
                      in1=rstd_all, op0=OP.mult, op1=OP.mult,
                  )

                  for jj, b in enumerate(pair):
                    i = j0 + jj
                    cls = b // NBLK_CLS
                    ps, pd = (cls >> 1) & 1, cls & 1
                    e0 = b * BLK
                    sl = slice(e0, e0 + BLK)
                    psmix = psmixes[b]
                    rstd = rstd_all[:, jj * SUB : (jj + 1) * SUB]
                    tb = tb_all[:, jj * SUB : (jj + 1) * SUB]

                    acc = accp.tile([P, SUB], F32, tag="acc")

                    # batched dT transposes: two sub-tiles of demb per transpose
                    dTs = []
                    for h in range(2):
                        dT_ps = ps_t8.tile([P, 8, P], BF16, tag="ptp8")
                        nc.tensor.transpose(
                            dT_ps[:, 0, :],
                            demb[:, (i * SUB + 2 * h) : (i * SUB + 2 * h + 2), :]
                            .rearrange("p s k -> p (s k)"),
                            identb[:],
                        )
                        dT = trsbp.tile([P, P], BF16, tag=f"dT{h}")
                        nc.scalar.copy(dT[:], dT_ps[:, 0, :])
                        dTs.append(dT)

                    # ---- LN apply for all sub-tiles up front (frees psmix) ----
                    ynorms = []
                    for s in range(SUB):
                        ynorm = workp.tile([P, NS], BF16, tag="ynorm")
                        nc.scalar.activation(
                            ynorm[:], psmix[:, s, :], AF.Identity,
                            bias=tb[:, s : s + 1], scale=rstd[:, s : s + 1],
                        )
                        ynorms.append(ynorm)

                    # ---- pass 2: dfilter + final MLP ----
                    for s in range(SUB):
                        ynorm = ynorms[s]
                        h0 = (s % 2) * NB
                        dT = dTs[s // 2][h0 : h0 + NB, :]
                        rhs1 = dfw1_sb[h0 : h0 + NB, :]
                        ph = ps_hd.tile([P, 128], F32, tag="ph")
                        if zero_bias:
                            nc.tensor.matmul(ph[:], lhsT=dT, rhs=rhs1, start=True, stop=True)
                        else:
                            nc.tensor.matmul(ph[:], lhsT=dT, rhs=rhs1, start=True, stop=False)
                            nc.tensor.matmul(ph[:], lhsT=ones_row[:], rhs=dfb1_sb[:], start=False, stop=True)
                        sact = workp.tile([P, 128], BF16, tag="sact")
                        if zero_bias:
                            nc.scalar.activation(
                                sact[:], ph[:], AF.Silu,
                                scale=env[:, i * SUB + s : i * SUB + s + 1],
                            )
                        else:
                            nc.scalar.activation(sact[:], ph[:], AF.Silu)
                        sT_ps = ps_t8.tile([P, 8, P], BF16, tag="ptp8")
                        nc.tensor.transpose(sT_ps[:, 0, :], sact[:], identb[:])
                        sT = trsbp.tile([P, P], BF16, tag="sT")
                        nc.vector.tensor_copy(sT[:], sT_ps[:, 0, :])
                        rg = workp.tile([P, 128], BF16, tag="rg")
                        if zero_bias:
                            pdf = ps_hd.tile([P, 128], F32, tag="pdf")
                            nc.tensor.matmul(pdf[:], lhsT=sT[:], rhs=dfw2gb_sb[:], start=True, stop=True)
                            nc.vector.tensor_mul(rg[:], ynorm[:], pdf[:])
                        else:
                            pdf = ps_hd.tile([P, 256], F32, tag="pdf")
                            nc.tensor.matmul(pdf[:], lhsT=sT[:], rhs=dfw2gb_sb[:], start=True, stop=False)
                            nc.tensor.matmul(pdf[:], lhsT=ones_row[:], rhs=dfb2gb_sb[:], start=False, stop=True)
                            nc.vector.tensor_mul(rg[:], ynorm[:], pdf[:, 0:128])
                            nc.vector.tensor_add(rg[:], rg[:], pdf[:, 128:256])

                        rT_ps = ps_t8.tile([P, 8, P], BF16, tag="ptp8")
                        nc.tensor.transpose(rT_ps[:, 0, :], rg[:], identb[:])
                        rT = trsbp.tile([P, P], BF16, tag="rT")
                        nc.scalar.copy(rT[:], rT_ps[:, 0, :])
                        pg2 = ps_g.tile([P, 512], F32, tag="pg")
                        if zero_bias:
                            nc.tensor.matmul(pg2[:], lhsT=rT[:], rhs=mlpw1_sb[:], start=True, stop=True)
                        else:
                            nc.tensor.matmul(pg2[:], lhsT=rT[:], rhs=mlpw1_sb[:], start=True, stop=False)
                            nc.tensor.matmul(pg2[:], lhsT=ones_row[:], rhs=mlpb1_sb[:], start=False, stop=True)
                        gact = workp.tile([P, 512], BF16, tag="gact")
                        nc.scalar.activation(gact[:], pg2[:], AF.Silu)
                        scr = workp.tile([P, 512], BF16, tag="scr")
                        nc.vector.scalar_tensor_tensor(
                            out=scr[:], in0=gact[:], scalar=1.0, in1=w2rep_sb[:],
                            op0=OP.mult, op1=OP.mult,
                            accum_out=acc[:, s : s + 1],
                        )

                    if not zero_bias:
                        nc.vector.tensor_scalar(
                            out=acc[:], in0=acc[:], scalar1=b2_sb[:, 0:1], scalar2=None,
                            op0=OP.add,
                        )
                    nc.sync.dma_start(out=out[sl].rearrange("(s p) -> p s", p=P), in_=acc[:])

    nc.compile()
    return nc


def _get_compiled(zero_bias: bool = True):
    if zero_bias not in _compiled:
        _compiled[zero_bias] = _build(NBLK, zero_bias)
    return _compiled[zero_bias]


def _wrap16(idx_block):
    """int array [512] -> dma_gather wrapped int16 layout [128, 32]
    (index j at [j%16, j//16], replicated across the 8 gpsimd cores)."""
    w = idx_block.astype(np.int16).reshape(-1, 16).T  # [16, n/16]
    return np.tile(w, (8, 1))


def _prep(inputs):
    nodes = np.asarray(inputs["nodes"], np.float32)
    edge_index = np.asarray(inputs["edge_index"]).astype(np.int64)
    graph_batch = np.asarray(inputs["graph_batch"]).astype(np.int64)
    cell = np.asarray(inputs["cell"], np.float32).reshape(32, 9)
    edge_shift = np.asarray(inputs["edge_shift"], np.float32)
    pos = np.asarray(inputs["pos"], np.float32)

    # bf16 pair-row node table with fp32 pos bit-packed at units 120:126
    row_u16 = np.zeros((N_NODES, XR), np.uint16)
    row_u16[:, 0:NODE_DIM] = nodes.astype(ml_dtypes.bfloat16).view(np.uint16)
    row_u16[:, 120:126] = pos.view(np.uint16).reshape(N_NODES, 6)
    nodes_pair = row_u16.reshape(NPAIR, 2 * XR).view(ml_dtypes.bfloat16)

    alpha = 1.0 / np.sqrt(float(L0 * L0 + L1 * L1 + L2 * L2))
    w0 = np.asarray(inputs["W0"], np.float32).reshape(L0 * L0, NS) * alpha
    w1 = np.asarray(inputs["W1"], np.float32).reshape(L1 * L1, NS) * (alpha / np.sqrt(3.0))
    w2 = np.asarray(inputs["W2"], np.float32).reshape(L2 * L2, NS) * (alpha / np.sqrt(5.0))
    wflat = np.zeros((KPAD, NS), np.float32)
    wflat[0:1024] = w0
    wflat[1024:1280] = w1
    wflat[1280:1344] = w2

    ln_g = np.asarray(inputs["ln_g"], np.float32)
    ln_b = np.asarray(inputs["ln_b"], np.float32)
    df_w2 = np.asarray(inputs["df_w2"], np.float32)
    df_b2 = np.asarray(inputs["df_b2"], np.float32)
    dfw2gb = np.concatenate([df_w2 * ln_g[None, :], df_w2 * ln_b[None, :]], axis=1)
    dfb2gb = np.concatenate([df_b2 * ln_g, df_b2 * ln_b])[None, :]

    zero_bias = (
        not np.any(np.asarray(inputs["df_b1"]))
        and not np.any(np.asarray(inputs["df_b2"]))
        and not np.any(np.asarray(inputs["mlp_b1"]))
        and not np.any(np.asarray(inputs["mlp_b2"]))
        and not np.any(ln_b)
    )

    bf = lambda a: np.ascontiguousarray(a).astype(ml_dtypes.bfloat16)

    common = {
        "nodes_pair": nodes_pair,
        "wflat": bf(wflat),
        "dfw1": bf(np.asarray(inputs["df_w1"], np.float32)),
        "dfb1": bf(np.asarray(inputs["df_b1"], np.float32)[None, :]),
        "dfw2gb": bf(dfw2gb),
        "dfb2gb": bf(dfb2gb),
        "mlpw1": bf(np.asarray(inputs["mlp_w1"], np.float32)),
        "mlpb1": bf(np.asarray(inputs["mlp_b1"], np.float32)[None, :]),
        "w2row": bf(np.asarray(inputs["mlp_w2"], np.float32).T),
        "b2sc": np.asarray(inputs["mlp_b2"], np.float32).reshape(1, 1),
        "offs": np.linspace(0.0, CUTOFF, NB, dtype=np.float32)[None, :],
    }

    in_maps = []
    outmaps = []
    for c in range(NCORES):
        lo, hi = c * E_CORE, (c + 1) * E_CORE
        src = edge_index[0, lo:hi]
        dst = edge_index[1, lo:hi]
        esh = edge_shift[lo:hi]
        key = ((src & 1) << 1) | (dst & 1)

        srcp = np.zeros(E_PAD, np.int64)
        dstp = np.zeros(E_PAD, np.int64)
        geo = np.zeros((E_PAD, 12), np.float32)
        outmap = np.full(E_PAD, -1, np.int64)
        for cls in range(4):
            idxs = np.nonzero(key == cls)[0]
            n = len(idxs)
            assert n <= ECLS, f"class {cls} overflow: {n} > {ECLS}"
            base = cls * ECLS
            srcp[base : base + n] = src[idxs]
            dstp[base : base + n] = dst[idxs]
            geo[base : base + n, 0:3] = esh[idxs]
            geo[base : base + n, 3:12] = cell[graph_batch[src[idxs]]]
            outmap[base : base + n] = idxs

        xw = np.zeros((NBLK, P, 64), np.int16)
        for b in range(NBLK):
            sb = srcp[b * BLK : (b + 1) * BLK]
            db = dstp[b * BLK : (b + 1) * BLK]
            xw[b, :, 0:32] = _wrap16(sb >> 1)
            xw[b, :, 32:64] = _wrap16(db >> 1)

        m = dict(common)
        m["xw16"] = xw
        m["geo12"] = geo
        in_maps.append(m)
        outmaps.append(outmap)
    return in_maps, outmaps, zero_bias


def _gather_out(res, outmaps, b2_host=0.0):
    full = np.empty((N_EDGES,), np.float32)
    for c in range(NCORES):
        dev = np.asarray(res.results[c]["out"])
        outmap = outmaps[c]
        valid = outmap >= 0
        full[c * E_CORE + outmap[valid]] = dev[valid]
    return full.reshape(N_EDGES, 1)


def kernel(**inputs) -> np.ndarray:
    in_maps, outmaps, zero_bias = _prep(inputs)
    nc = _get_compiled(zero_bias)
    res = run_bass_kernel_spmd(nc, in_maps, core_ids=list(range(NCORES)))
    return _gather_out(res, outmaps)


# revision 38
# speedup vs baseline: 1.1126x; 1.0873x over previous
"""Trainium2 Bass kernel for nn_ExchangeBlock (GNN message passing / e3nn-style
tensor-product edge block), SPMD across 8 NeuronCores.

Sharding: edges across the 8 cores; node features and params replicated.

v4 design notes:
- Edges are host-sorted into 4 parity classes (src&1, dst&1) so the pair-row
  parity select becomes a compile-time AP slice: no masks, no predicated
  copies, no gpsimd copy traffic at all.
- ONE dma_gather per block: the node table is bf16 pair rows (512B) with the
  fp32 position bit-packed into units 120:126 of each row, so geometry and
  the tensor product share the same gathered tile.  cell[graph_batch[src]]
  is a host-side index prep (like the baseline's graph_batch[src]) and is
  streamed per edge together with edge_shift.
- Blocks run in groups of 8 with two ScalarE activation-table phases per
  group (exp set: RBF; silu set: MLPs).  The per-edge scalar chain (Newton
  rsqrt, cutoff envelope as an exact degree-6 polynomial in d^2, RBF, demb)
  is batched once per group, so phase A is a handful of wide ops and the
  Square/Exp are single instructions the OoO scheduler cannot shred.
- The TP runs as outer-product features P[e,1344] built on DVE (bf16), PE
  transposes of P chunks (one full PSUM bank per 8 chunks), and accumulated
  128x128 matmuls against pre-scaled flattened weights.  The 1o/2e paths
  build m-major product tiles and reduce with 2 contiguous bf16 adds
  instead of a slow innermost-3 reduce.
- LN stats via one PSUM reduce + a batched ScalarE Square + one bf16
  reduce; the final w2 contraction fuses multiply+reduce into one
  scalar_tensor_tensor with accum_out.
- All MLP biases in this problem are exactly zero; _prep detects that and
  compiles the bias-free variant (rank-1 PE bias matmuls otherwise).
"""

import sys

sys.path.insert(0, "/opt/trn_rl_repo")

import numpy as np
import ml_dtypes

import concourse.bass as bass
import concourse.mybir as mybir
import concourse.tile as tile
from concourse import bacc
from concourse.bass_utils import run_bass_kernel_spmd
from concourse.masks import make_identity

F32 = mybir.dt.float32
BF16 = mybir.dt.bfloat16
I32 = mybir.dt.int32
I16 = mybir.dt.int16
AF = mybir.ActivationFunctionType
OP = mybir.AluOpType

# Problem constants
L0, L1, L2 = 32, 16, 8
NS = 128
NB = 64
CUTOFF = 7.0
N_NODES = 50000
N_EDGES = 400000
NODE_DIM = 120
NCORES = 8

BLK = 512             # edges per block
SUB = 4               # 128-edge sub-tiles per block
P = 128
KTP = 1344            # 1024 + 256 + 64 contraction size
KPAD = 1408           # padded to 11 chunks of 128
NCHUNK = 11
RSQRT_MAGIC = 0x5F3759DF
NPAIR = N_NODES // 2  # 25000
XR = 128              # bf16 units per node row (120 nodes + 6 pos-halves + 2 pad)

E_CORE = N_EDGES // NCORES                      # 50000
ECLS = 13312                                    # padded edges per parity class
NBLK_CLS = ECLS // BLK                          # 26
NBLK = 4 * NBLK_CLS                             # 104
E_PAD = NBLK * BLK                              # 53248
GROUP = 13                                      # blocks per act-table phase group
GS = GROUP * SUB                                # 32 sub-tiles per group

# cos(pi/2 * sqrt(t)) Taylor coefficients, t = min(d^2/49, 1)
ENV_A = (
    1.0,
    -1.2337005500358182,
    0.25366950654487275,
    -0.020863473217859734,
    0.0009192394784838294,
    -2.5171984603292395e-05,
    4.492184960014096e-07,
)

_compiled = {}


def _patch_walrus_dge_levels():
    """This walrus build compiles with DynamicDMA disabled by default, which
    makes dynamic-offset DMAs crash the exec unit. Append the full
    --dge-levels set to every walrus invocation."""
    import concourse.bass_utils as _bu

    if getattr(_bu, "_dge_patched", False):
        return
    orig = _bu.run_command

    def patched(argv, **kw):
        if argv and "walrus_driver" in str(argv[0]) and not any(
            "dge-levels" in str(a) for a in argv
        ):
            argv = list(argv) + [
                "--dge-levels=io,spill_reload,scalar_dynamic_offset,"
                "vector_dynamic_offsets,dynamic_size,dst_reduce,transpose"
            ]
        return orig(argv, **kw)

    _bu.run_command = patched
    _bu._dge_patched = True


_patch_walrus_dge_levels()


def _patch_drain_and_barrier():
    """The final Tile drain runs on the SP engine, whose Drain lowering in this
    walrus build has no free sync-wait slots (its HWDGE queue waits fill them).
    Hoist the tile-clock waits onto dedicated nop instructions emitted just
    before the drain, one wait per nop."""
    if getattr(tile.TileContext, "_dab_patched", False):
        return

    def patched(self, tick_clock, wait_clock):
        nc = self.nc
        nops = [nc.sync.nop() for _ in range(32)]
        drain_inst = nc.sync.drain()
        from concourse.tile import ScopedClock

        wait_clock.add_sem_waits(
            drain_inst.ins, ScopedClock({None: tick_clock.global_clock})
        )
        si = drain_inst.ins.sync_info
        waits = list(si.on_wait) if si and si.on_wait else []
        if waits:
            assert len(waits) <= len(nops), f"{len(waits)} waits > nop slots"
            si.on_wait = []
            for w, n in zip(waits, nops):
                n.ins.sync_info = mybir.SyncInfo(on_wait=[w], on_update=[])

        nc.all_engine_barrier()
        assert self.sems is not None
        popped = nc._tile_sem_poison_stack.pop()
        assert popped is self._sem_poison
        nc.clear_and_free_semaphores(list(self.sems.allocated().values()))
        nc.all_engine_barrier()

    tile.TileContext._drain_and_barrier = patched
    tile.TileContext._dab_patched = True


_patch_drain_and_barrier()


def _newton_rsqrt(nc, pool, u, n, magic_t, tag, iters=3):
    """rsqrt(u) for u[:, :n] > 0 on the VectorEngine (no ScalarE table)."""
    bits = pool.tile([P, n], I32, tag=f"{tag}_b")
    nc.vector.tensor_copy(out=bits[:].bitcast(F32), in_=u)  # raw bit copy
    nc.vector.tensor_scalar(
        out=bits[:], in0=bits[:], scalar1=1, scalar2=None,
        op0=OP.arith_shift_right,
    )
    yb = pool.tile([P, n], I32, tag=f"{tag}_y")
    nc.vector.tensor_tensor(
        out=yb[:], in0=magic_t[:, 0:1].to_broadcast([P, n]), in1=bits[:],
        op=OP.subtract,
    )
    y = yb[:].bitcast(F32)
    t1 = pool.tile([P, n], F32, tag=f"{tag}_t1")
    for _ in range(iters):
        nc.vector.tensor_mul(t1[:], y, y)
        nc.vector.tensor_mul(t1[:], t1[:], u)
        nc.vector.tensor_scalar(
            out=t1[:], in0=t1[:], scalar1=-0.5, scalar2=1.5, op0=OP.mult, op1=OP.add,
        )
        nc.vector.tensor_mul(y, y, t1[:])
    return yb


def _build(nblocks: int, zero_bias: bool):
    nc = bacc.Bacc("TRN2", target_bir_lowering=False, debug=False)

    nodes_pair = nc.dram_tensor("nodes_pair", (NPAIR, 2 * XR), BF16, kind="ExternalInput").ap()
    xw16 = nc.dram_tensor("xw16", (nblocks, P, 64), I16, kind="ExternalInput").ap()
    geo12 = nc.dram_tensor("geo12", (nblocks * BLK, 12), F32, kind="ExternalInput").ap()
    wflat = nc.dram_tensor("wflat", (KPAD, NS), BF16, kind="ExternalInput").ap()
    dfw1 = nc.dram_tensor("dfw1", (NB, 128), BF16, kind="ExternalInput").ap()
    dfb1 = nc.dram_tensor("dfb1", (1, 128), BF16, kind="ExternalInput").ap()
    dfw2gb = nc.dram_tensor("dfw2gb", (128, 256), BF16, kind="ExternalInput").ap()
    dfb2gb = nc.dram_tensor("dfb2gb", (1, 256), BF16, kind="ExternalInput").ap()
    mlpw1 = nc.dram_tensor("mlpw1", (128, 512), BF16, kind="ExternalInput").ap()
    mlpb1 = nc.dram_tensor("mlpb1", (1, 512), BF16, kind="ExternalInput").ap()
    w2row = nc.dram_tensor("w2row", (1, 512), BF16, kind="ExternalInput").ap()
    b2sc = nc.dram_tensor("b2sc", (1, 1), F32, kind="ExternalInput").ap()
    offs = nc.dram_tensor("offs", (1, NB), F32, kind="ExternalInput").ap()
    out = nc.dram_tensor("out", (nblocks * BLK,), F32, kind="ExternalOutput").ap()

    width = CUTOFF / (NB - 1)
    coeff = 0.5 / (width * width)
    sqc = float(np.sqrt(coeff))

    XGBUFS = GROUP + 6

    with tile.TileContext(nc) as tc:
        with (
            tc.tile_pool(name="const", bufs=1) as constp,
            tc.tile_pool(name="xgp", bufs=XGBUFS) as xgp,
            tc.tile_pool(name="grp", bufs=3) as grpp,
            tc.tile_pool(name="rbp", bufs=2) as rbp,
            tc.tile_pool(name="io", bufs=6) as iop,
            tc.tile_pool(name="geo", bufs=4) as geop,
            tc.tile_pool(name="pfeat", bufs=3) as pfp,
            tc.tile_pool(name="trsb", bufs=4) as trsbp,
            tc.tile_pool(name="work", bufs=4) as workp,
            tc.tile_pool(name="acc", bufs=3) as accp,
            tc.tile_pool(name="ps_t8", bufs=3, space="PSUM") as ps_t8,
            tc.tile_pool(name="ps_mm", bufs=2, space="PSUM") as ps_mm,
            tc.tile_pool(name="ps_hd", bufs=1, space="PSUM") as ps_hd,
            tc.tile_pool(name="ps_g", bufs=1, space="PSUM") as ps_g,
        ):
            # ---- resident constants ----
            identb = constp.tile([P, P], BF16)
            make_identity(nc, identb[:])
            magic_t = constp.tile([P, 1], I32)
            nc.vector.memset(magic_t[:], RSQRT_MAGIC)
            ones_row = constp.tile([1, P], BF16)
            nc.vector.memset(ones_row[:], 1.0)

            w_sb = constp.tile([P, NCHUNK, P], BF16)
            nc.sync.dma_start(out=w_sb[:], in_=wflat.rearrange("(c p) w -> p c w", p=P))
            dfw1_sb = constp.tile([P, 128], BF16)
            nc.sync.dma_start(out=dfw1_sb[0:NB, :], in_=dfw1)
            nc.sync.dma_start(out=dfw1_sb[NB:P, :], in_=dfw1)
            if zero_bias:
                dfw2gb_sb = constp.tile([128, 128], BF16)
                nc.sync.dma_start(out=dfw2gb_sb[:], in_=dfw2gb[:, 0:128])
            else:
                dfw2gb_sb = constp.tile([128, 256], BF16)
                nc.sync.dma_start(out=dfw2gb_sb[:], in_=dfw2gb)
            mlpw1_sb = constp.tile([128, 512], BF16)
            nc.sync.dma_start(out=mlpw1_sb[:], in_=mlpw1)
            w2rep_sb = constp.tile([P, 512], BF16)
            nc.sync.dma_start(out=w2rep_sb[:], in_=w2row.to_broadcast([P, 512]))
            b2_sb = constp.tile([P, 1], F32)
            nc.sync.dma_start(out=b2_sb[:], in_=b2sc.to_broadcast([P, 1]))
            offs_sb = constp.tile([P, NB], F32)
            nc.sync.dma_start(out=offs_sb[:], in_=offs.to_broadcast([P, NB]))
            if not zero_bias:
                dfb1_sb = constp.tile([1, 128], BF16)
                nc.sync.dma_start(out=dfb1_sb[:], in_=dfb1)
                dfb2gb_sb = constp.tile([1, 256], BF16)
                nc.sync.dma_start(out=dfb2gb_sb[:], in_=dfb2gb)
                mlpb1_sb = constp.tile([1, 512], BF16)
                nc.sync.dma_start(out=mlpb1_sb[:], in_=mlpb1)

            xg_tiles = {}

            groups = [range(g, min(g + GROUP, nblocks)) for g in range(0, nblocks, GROUP)]
            for blocks in groups:
                gn = len(blocks)
                g0 = blocks[0]
                # ======== Phase A: gather + geometry + RBF (exp table) ========
                d2g = grpp.tile([P, GS], F32, tag="d2g")
                geog = grpp.tile([P, GROUP, SUB, 12], F32, tag="geog")
                nc.sync.dma_start(
                    out=geog[:, 0:gn, :, :],
                    in_=geo12[g0 * BLK : (g0 + gn) * BLK, :].rearrange(
                        "(g s p) j -> p g s j", p=P, s=SUB
                    ),
                )
                tvp = grpp.tile([P, GROUP, SUB, 3, 3], F32, tag="tvp")
                nc.vector.tensor_tensor(
                    out=tvp[:, 0:gn],
                    in0=geog[:, 0:gn, :, 0:3].unsqueeze(4).to_broadcast([P, gn, SUB, 3, 3]),
                    in1=geog[:, 0:gn, :, 3:12].rearrange("p g s (i j) -> p g s i j", j=3),
                    op=OP.mult,
                )
                tvg = grpp.tile([P, GROUP, SUB, 3], F32, tag="tvg")
                nc.vector.reduce_sum(
                    out=tvg[:, 0:gn], in_=tvp[:, 0:gn].transpose([0, 1, 2, 4, 3]),
                    axis=mybir.AxisListType.X,
                )
                for i, b in enumerate(blocks):
                    cls = b // NBLK_CLS
                    ps, pd = (cls >> 1) & 1, cls & 1

                    xw = iop.tile([P, 64], I16, tag="xw")
                    nc.sync.dma_start(out=xw[:], in_=xw16[b])
                    xg = xgp.tile([P, 2 * SUB, 2 * XR], BF16, tag="xg")
                    nc.gpsimd.dma_gather(
                        out_ap=xg[:], in_ap=nodes_pair[:, :], idxs_ap=xw[:],
                        num_idxs=2 * BLK, num_idxs_reg=2 * BLK, elem_size=2 * XR,
                    )
                    xg_tiles[b] = xg

                    # fp32 positions bit-packed into the bf16 rows
                    p1 = xg[:, 0:SUB, ps * XR + 120 : ps * XR + 126].bitcast(F32)
                    p2 = xg[:, SUB : 2 * SUB, pd * XR + 120 : pd * XR + 126].bitcast(F32)

                    rv = geop.tile([P, SUB, 3], F32, tag="rv")
                    nc.vector.tensor_sub(rv[:], p2, p1)
                    nc.vector.tensor_add(rv[:], rv[:], tvg[:, i])
                    rv2 = geop.tile([P, SUB, 3], F32, tag="rv2")
                    nc.vector.tensor_mul(rv2[:], rv[:], rv[:])
                    nc.vector.reduce_sum(
                        out=d2g[:, i * SUB : (i + 1) * SUB], in_=rv2[:],
                        axis=mybir.AxisListType.X,
                    )

                ng = gn * SUB
                nc.vector.tensor_scalar(
                    out=d2g[:, 0:ng], in0=d2g[:, 0:ng], scalar1=1e-12, scalar2=None,
                    op0=OP.max,
                )
                ry = _newton_rsqrt(nc, grpp, d2g[:, 0:ng], ng, magic_t, "rsq", iters=2)
                dist = grpp.tile([P, GS], F32, tag="dist")
                nc.vector.tensor_mul(dist[:, 0:ng], d2g[:, 0:ng], ry[:].bitcast(F32))

                # envelope: env = p(t)^2, t = min(d2/49, 1)
                tgeo = grpp.tile([P, GS], F32, tag="tgeo")
                nc.vector.tensor_scalar(
                    out=tgeo[:, 0:ng], in0=d2g[:, 0:ng], scalar1=1.0 / 49.0, scalar2=1.0,
                    op0=OP.mult, op1=OP.min,
                )
                envr = grpp.tile([P, GS], F32, tag="envr")
                nc.vector.tensor_scalar(
                    out=envr[:, 0:ng], in0=tgeo[:, 0:ng], scalar1=ENV_A[6], scalar2=None,
                    op0=OP.mult,
                )
                for k in range(5, 0, -1):
                    nc.vector.scalar_tensor_tensor(
                        out=envr[:, 0:ng], in0=envr[:, 0:ng], scalar=ENV_A[k],
                        in1=tgeo[:, 0:ng], op0=OP.add, op1=OP.mult,
                    )
                env = grpp.tile([P, GS], F32, tag="env")
                nc.vector.tensor_scalar(
                    out=env[:, 0:ng], in0=envr[:, 0:ng], scalar1=ENV_A[0], scalar2=None,
                    op0=OP.add,
                )
                nc.vector.tensor_mul(env[:, 0:ng], env[:, 0:ng], env[:, 0:ng])

                # rbf then demb = rbf * env (one Square + one Exp per group)
                rb = rbp.tile([P, GS, NB], F32, tag="rb")
                nc.vector.tensor_tensor(
                    out=rb[:, 0:ng, :],
                    in0=offs_sb[:].unsqueeze(1).to_broadcast([P, ng, NB]),
                    in1=dist[:, 0:ng].unsqueeze(2).to_broadcast([P, ng, NB]),
                    op=OP.subtract,
                )
                nc.scalar.activation(rb[:, 0:ng, :], rb[:, 0:ng, :], AF.Square, scale=sqc)
                demb = grpp.tile([P, GS, NB], BF16, tag="demb")
                if zero_bias:
                    # env is folded into the dfilter silu scale downstream
                    nc.scalar.activation(demb[:, 0:ng, :], rb[:, 0:ng, :], AF.Exp, scale=-1.0)
                else:
                    nc.scalar.activation(rb[:, 0:ng, :], rb[:, 0:ng, :], AF.Exp, scale=-1.0)
                    nc.vector.tensor_tensor(
                        out=demb[:, 0:ng, :], in0=rb[:, 0:ng, :],
                        in1=env[:, 0:ng].unsqueeze(2).to_broadcast([P, ng, NB]),
                        op=OP.mult,
                    )

                # ======== Phase B: TP + LN + dfilter + MLP (silu table) ========
                for j0 in range(0, gn, 2):
                  pair = list(blocks[j0 : j0 + 2])
                  np_pair = len(pair)
                  sumv = geop.tile([P, 2, SUB], F32, tag="sumv")
                  sumsq = geop.tile([P, 2, SUB], F32, tag="sumsq")
                  psmixes = {}
                  for jj, b in enumerate(pair):
                    i = j0 + jj
                    cls = b // NBLK_CLS
                    ps, pd = (cls >> 1) & 1, cls & 1
                    xg = xg_tiles.pop(b)

                    x1 = xg[:, 0:SUB, ps * XR : ps * XR + 120]
                    x2 = xg[:, SUB : 2 * SUB, pd * XR : pd * XR + 120]

                    psmix = ps_mm.tile([P, SUB, NS], F32, tag="psmix")
                    psmixes[b] = psmix

                    # ---- pass 1: tensor product per sub-tile ----
                    for s in range(SUB):
                        ptb0 = pfp.tile([P, 1024], BF16, tag="ptb0")
                        ptb1 = pfp.tile([P, 384], BF16, tag="ptb1")
                        a1 = x1[:, s, 0:L0]
                        a2 = x2[:, s, 0:L0]
                        nc.vector.tensor_tensor(
                            out=ptb0[:].rearrange("p (u v) -> p u v", v=L0),
                            in0=a1.unsqueeze(2).to_broadcast([P, L0, L0]),
                            in1=a2.unsqueeze(1).to_broadcast([P, L0, L0]),
                            op=OP.mult,
                        )
                        # 0e transposes + copy + matmuls depend only on ptb0,
                        # overlapping the 1o/2e build below
                        ptp8 = ps_t8.tile([P, 8, P], BF16, tag="ptp8")
                        for c in range(8):
                            nc.tensor.transpose(
                                ptp8[:, c, :], ptb0[:, c * P : (c + 1) * P], identb[:]
                            )
                        pts8 = trsbp.tile([P, 8, P], BF16, tag="pts8")
                        nc.scalar.copy(pts8[:], ptp8[:])
                        for c in range(8):
                            nc.tensor.matmul(
                                psmix[:, s, :], lhsT=pts8[:, c, :], rhs=w_sb[:, c, :],
                                start=(c == 0), stop=False,
                            )
                        # 1o path: m-major product tile, reduce via 2 contiguous adds
                        b1 = x1[:, s, 32:80].rearrange("p (u m) -> p m u", m=3)
                        b2 = x2[:, s, 32:80].rearrange("p (v m) -> p m v", m=3)
                        pb = workp.tile([P, 3, L1, L1], BF16, tag="pb")
                        nc.vector.tensor_tensor(
                            out=pb[:],
                            in0=b1.unsqueeze(3).to_broadcast([P, 3, L1, L1]),
                            in1=b2.unsqueeze(2).to_broadcast([P, 3, L1, L1]),
                            op=OP.mult,
                        )
                        pbf = pb[:].rearrange("p m u v -> p m (u v)")
                        with nc.allow_low_precision(reason="3-term bf16 add"):
                            t01 = workp.tile([P, L1 * L1], BF16, tag="t01")
                            nc.vector.tensor_add(t01[:], pbf[:, 0, :], pbf[:, 1, :])
                            nc.vector.tensor_add(ptb1[:, 0:256], t01[:], pbf[:, 2, :])
                        # 2e path
                        c1 = x1[:, s, 80:120].rearrange("p (u m) -> p m u", m=5)
                        c2 = x2[:, s, 80:120].rearrange("p (v m) -> p m v", m=5)
                        pc = workp.tile([P, 5, L2, L2], BF16, tag="pc")
                        nc.vector.tensor_tensor(
                            out=pc[:],
                            in0=c1.unsqueeze(3).to_broadcast([P, 5, L2, L2]),
                            in1=c2.unsqueeze(2).to_broadcast([P, 5, L2, L2]),
                            op=OP.mult,
                        )
                        pcf = pc[:].rearrange("p m u v -> p m (u v)")
                        with nc.allow_low_precision(reason="5-term bf16 add"):
                            u01 = workp.tile([P, L2 * L2], BF16, tag="u01")
                            nc.vector.tensor_add(u01[:], pcf[:, 0, :], pcf[:, 1, :])
                            u23 = workp.tile([P, L2 * L2], BF16, tag="u23")
                            nc.vector.tensor_add(u23[:], pcf[:, 2, :], pcf[:, 3, :])
                            nc.vector.tensor_add(u01[:], u01[:], u23[:])
                            nc.vector.tensor_add(ptb1[:, 256:320], u01[:], pcf[:, 4, :])

                        ptp4 = ps_t8.tile([P, 8, P], BF16, tag="ptp8")
                        nc.tensor.transpose(ptp4[:, 0, :], ptb1[:, 0:128], identb[:])
                        nc.tensor.transpose(ptp4[:, 1, :], ptb1[:, 128:256], identb[:])
                        nc.tensor.transpose(ptp4[0:64, 2, :], ptb1[:, 256:320], identb[:])
                        pts4 = trsbp.tile([P, 4, P], BF16, tag="pts4")
                        nc.scalar.copy(pts4[:, 0:3, :], ptp4[:, 0:3, :])
                        nc.tensor.matmul(
                            psmix[:, s, :], lhsT=pts4[:, 0, :], rhs=w_sb[:, 8, :],
                            start=False, stop=False,
                        )
                        nc.tensor.matmul(
                            psmix[:, s, :], lhsT=pts4[:, 1, :], rhs=w_sb[:, 9, :],
                            start=False, stop=False,
                        )
                        nc.tensor.matmul(
                            psmix[:, s, :], lhsT=pts4[0:64, 2, :], rhs=w_sb[0:64, 10, :],
                            start=False, stop=True,
                        )

                    # per-block LN stat reduces into the pair tile
                    nc.vector.reduce_sum(
                        out=sumv[:, jj, :], in_=psmix[:], axis=mybir.AxisListType.X,
                    )
                    sq = workp.tile([P, SUB, NS], BF16, tag="sq")
                    nc.scalar.activation(sq[:], psmix[:], AF.Square)
                    with nc.allow_low_precision(reason="bf16 sumsq reduce"):
                        nc.vector.reduce_sum(
                            out=sumsq[:, jj, :], in_=sq[:], axis=mybir.AxisListType.X,
                        )

                  # ---- pair-level LN scalar chain ----
                  nsx = np_pair * SUB
                  sumvf = sumv[:].rearrange("p j s -> p (j s)")
                  sumsqf = sumsq[:].rearrange("p j s -> p (j s)")
                  muv = geop.tile([P, 2 * SUB], F32, tag="muv")
                  nc.vector.tensor_scalar(
                      out=muv[:, 0:nsx], in0=sumvf[:, 0:nsx], scalar1=1.0 / NS,
                      scalar2=None, op0=OP.mult,
                  )
                  varv = geop.tile([P, 2 * SUB], F32, tag="varv")
                  nc.vector.tensor_mul(varv[:, 0:nsx], muv[:, 0:nsx], muv[:, 0:nsx])
                  nc.vector.scalar_tensor_tensor(
                      out=varv[:, 0:nsx], in0=sumsqf[:, 0:nsx], scalar=1.0 / NS,
                      in1=varv[:, 0:nsx], op0=OP.mult, op1=OP.subtract,
                  )
                  nc.vector.tensor_scalar(
                      out=varv[:, 0:nsx], in0=varv[:, 0:nsx], scalar1=1e-5,
                      scalar2=None, op0=OP.add,
                  )
                  ryl = _newton_rsqrt(nc, geop, varv[:, 0:nsx], nsx, magic_t, "lnr", iters=2)
                  rstd_all = ryl[:].bitcast(F32)
                  tb_all = geop.tile([P, 2 * SUB], F32, tag="tb")
                  nc.vector.scalar_tensor_tensor(
                      out=tb_all[:, 0:nsx], in0=muv[:, 0:nsx], scalar=-1.0,
                      in1=rstd_all, op0=OP.mult, op1=OP.mult,
                  )

                  for jj, b in enumerate(pair):
                    i = j0 + jj
                    cls = b // NBLK_CLS
                    ps, pd = (cls >> 1) & 1, cls & 1
                    e0 = b * BLK
                    sl = slice(e0, e0 + BLK)
                    psmix = psmixes[b]
                    rstd = rstd_all[:, jj * SUB : (jj + 1) * SUB]
                    tb = tb_all[:, jj * SUB : (jj + 1) * SUB]

                    acc = accp.tile([P, SUB], F32, tag="acc")

                    # batched dT transposes: two sub-tiles of demb per transpose
                    dTs = []
                    for h in range(2):
                        dT_ps = ps_t8.tile([P, 8, P], BF16, tag="ptp8")
                        nc.tensor.transpose(
                            dT_ps[:, 0, :],
                            demb[:, (i * SUB + 2 * h) : (i * SUB + 2 * h + 2), :]
                            .rearrange("p s k -> p (s k)"),
                            identb[:],
                        )
                        dT = trsbp.tile([P, P], BF16, tag=f"dT{h}")
                        nc.scalar.copy(dT[:], dT_ps[:, 0, :])
                        dTs.append(dT)

                    # ---- LN apply for all sub-tiles up front (frees psmix) ----
                    ynorms = []
                    for s in range(SUB):
                        ynorm = workp.tile([P, NS], BF16, tag="ynorm")
                        nc.scalar.activation(
                            ynorm[:], psmix[:, s, :], AF.Identity,
                            bias=tb[:, s : s + 1], scale=rstd[:, s : s + 1],
                        )
                        ynorms.append(ynorm)

                    # ---- pass 2: dfilter + final MLP ----
                    for s in range(SUB):
                        ynorm = ynorms[s]
                        h0 = (s % 2) * NB
                        dT = dTs[s // 2][h0 : h0 + NB, :]
                        rhs1 = dfw1_sb[h0 : h0 + NB, :]
                        ph = ps_hd.tile([P, 128], F32, tag="ph")
                        if zero_bias:
                            nc.tensor.matmul(ph[:], lhsT=dT, rhs=rhs1, start=True, stop=True)
                        else:
                            nc.tensor.matmul(ph[:], lhsT=dT, rhs=rhs1, start=True, stop=False)
                            nc.tensor.matmul(ph[:], lhsT=ones_row[:], rhs=dfb1_sb[:], start=False, stop=True)
                        sact = workp.tile([P, 128], BF16, tag="sact")
                        if zero_bias:
                            nc.scalar.activation(
                                sact[:], ph[:], AF.Silu,
                                scale=env[:, i * SUB + s : i * SUB + s + 1],
                            )
                        else:
                            nc.scalar.activation(sact[:], ph[:], AF.Silu)
                        sT_ps = ps_t8.tile([P, 8, P], BF16, tag="ptp8")
                        nc.tensor.transpose(sT_ps[:, 0, :], sact[:], identb[:])
                        sT = trsbp.tile([P, P], BF16, tag="sT")
                        nc.vector.tensor_copy(sT[:], sT_ps[:, 0, :])
                        rg = workp.tile([P, 128], BF16, tag="rg")
                        if zero_bias:
                            pdf = ps_hd.tile([P, 128], F32, tag="pdf")
                            nc.tensor.matmul(pdf[:], lhsT=sT[:], rhs=dfw2gb_sb[:], start=True, stop=True)
                            nc.vector.tensor_mul(rg[:], ynorm[:], pdf[:])
                        else:
                            pdf = ps_hd.tile([P, 256], F32, tag="pdf")
                            nc.tensor.matmul(pdf[:], lhsT=sT[:], rhs=dfw2gb_sb[:], start=True, stop=False)
                            nc.tensor.matmul(pdf[:], lhsT=ones_row[:], rhs=dfb2gb_sb[:], start=False, stop=True)
                            nc.vector.tensor_mul(rg[:], ynorm[:], pdf[:, 0:128])
                            nc.vector.tensor_add(rg[:], rg[:], pdf[:, 128:256])

                        rT_ps = ps_t8.tile([P, 8, P], BF16, tag="ptp8")
                        nc.tensor.transpose(rT_ps[:, 0, :], rg[:], identb[:])
                        rT = trsbp.tile([P, P], BF16, tag="rT")
                        nc.scalar.copy(rT[:], rT_ps[:, 0, :])
                        pg2 = ps_g.tile([P, 512], F32, tag="pg")
                        if zero_bias:
                            nc.tensor.matmul(pg2[:], lhsT=rT[:], rhs=mlpw1_sb[:], start=True, stop=True)
                        else:
                            nc.tensor.matmul(pg2[:], lhsT=rT[:], rhs=mlpw1_sb[:], start=True, stop=False)
                            nc.tensor.matmul(pg2[:], lhsT=ones_row[:], rhs=mlpb1_sb[:], start=False, stop=True)
                        gact = workp.tile([P, 512], BF16, tag="gact")
                        nc.scalar.activation(gact[:], pg2[:], AF.Silu)
                        scr = workp.tile([P, 512], BF16, tag="scr")
                        nc.vector.scalar_tensor_tensor(
                            out=scr[:], in0=gact[:], scalar=1.0, in1=w2rep_sb[:],
                            op0=OP.mult, op1=OP.mult,
                            accum_out=acc[:, s : s + 1],
                        )

                    if not zero_bias:
                        nc.vector.tensor_scalar(
                            out=acc[:], in0=acc[:], scalar1=b2_sb[:, 0:1], scalar2=None,
                            op0=OP.add,
                        )
                    nc.sync.dma_start(out=out[sl].rearrange("(s p) -> p s", p=P), in_=acc[:])

    nc.compile()
    return nc


def _get_compiled(zero_bias: bool = True):
    if zero_bias not in _compiled:
        _compiled[zero_bias] = _build(NBLK, zero_bias)
    return _compiled[zero_bias]


def _wrap16(idx_block):
    """int array [512] -> dma_gather wrapped int16 layout [128, 32]
    (index j at [j%16, j//16], replicated across the 8 gpsimd cores)."""
    w = idx_block.astype(np.int16).reshape(-1, 16).T  # [16, n/16]
    return np.tile(w, (8, 1))


def _prep(inputs):
    nodes = np.asarray(inputs["nodes"], np.float32)
    edge_index = np.asarray(inputs["edge_index"]).astype(np.int64)
    graph_batch = np.asarray(inputs["graph_batch"]).astype(np.int64)
    cell = np.asarray(inputs["cell"], np.float32).reshape(32, 9)
    edge_shift = np.asarray(inputs["edge_shift"], np.float32)
    pos = np.asarray(inputs["pos"], np.float32)

    # bf16 pair-row node table with fp32 pos bit-packed at units 120:126
    row_u16 = np.zeros((N_NODES, XR), np.uint16)
    row_u16[:, 0:NODE_DIM] = nodes.astype(ml_dtypes.bfloat16).view(np.uint16)
    row_u16[:, 120:126] = pos.view(np.uint16).reshape(N_NODES, 6)
    nodes_pair = row_u16.reshape(NPAIR, 2 * XR).view(ml_dtypes.bfloat16)

    alpha = 1.0 / np.sqrt(float(L0 * L0 + L1 * L1 + L2 * L2))
    w0 = np.asarray(inputs["W0"], np.float32).reshape(L0 * L0, NS) * alpha
    w1 = np.asarray(inputs["W1"], np.float32).reshape(L1 * L1, NS) * (alpha / np.sqrt(3.0))
    w2 = np.asarray(inputs["W2"], np.float32).reshape(L2 * L2, NS) * (alpha / np.sqrt(5.0))
    wflat = np.zeros((KPAD, NS), np.float32)
    wflat[0:1024] = w0
    wflat[1024:1280] = w1
    wflat[1280:1344] = w2

    ln_g = np.asarray(inputs["ln_g"], np.float32)
    ln_b = np.asarray(inputs["ln_b"], np.float32)
    df_w2 = np.asarray(inputs["df_w2"], np.float32)
    df_b2 = np.asarray(inputs["df_b2"], np.float32)
    dfw2gb = np.concatenate([df_w2 * ln_g[None, :], df_w2 * ln_b[None, :]], axis=1)
    dfb2gb = np.concatenate([df_b2 * ln_g, df_b2 * ln_b])[None, :]

    zero_bias = (
        not np.any(np.asarray(inputs["df_b1"]))
        and not np.any(np.asarray(inputs["df_b2"]))
        and not np.any(np.asarray(inputs["mlp_b1"]))
        and not np.any(np.asarray(inputs["mlp_b2"]))
        and not np.any(ln_b)
    )

    bf = lambda a: np.ascontiguousarray(a).astype(ml_dtypes.bfloat16)

    common = {
        "nodes_pair": nodes_pair,
        "wflat": bf(wflat),
        "dfw1": bf(np.asarray(inputs["df_w1"], np.float32)),
        "dfb1": bf(np.asarray(inputs["df_b1"], np.float32)[None, :]),
        "dfw2gb": bf(dfw2gb),
        "dfb2gb": bf(dfb2gb),
        "mlpw1": bf(np.asarray(inputs["mlp_w1"], np.float32)),
        "mlpb1": bf(np.asarray(inputs["mlp_b1"], np.float32)[None, :]),
        "w2row": bf(np.asarray(inputs["mlp_w2"], np.float32).T),
        "b2sc": np.asarray(inputs["mlp_b2"], np.float32).reshape(1, 1),
        "offs": np.linspace(0.0, CUTOFF, NB, dtype=np.float32)[None, :],
    }

    in_maps = []
    outmaps = []
    for c in range(NCORES):
        lo, hi = c * E_CORE, (c + 1) * E_CORE
        src = edge_index[0, lo:hi]
        dst = edge_index[1, lo:hi]
        esh = edge_shift[lo:hi]
        key = ((src & 1) << 1) | (dst & 1)

        srcp = np.zeros(E_PAD, np.int64)
        dstp = np.zeros(E_PAD, np.int64)
        geo = np.zeros((E_PAD, 12), np.float32)
        outmap = np.full(E_PAD, -1, np.int64)
        for cls in range(4):
            idxs = np.nonzero(key == cls)[0]
            n = len(idxs)
            assert n <= ECLS, f"class {cls} overflow: {n} > {ECLS}"
            base = cls * ECLS
            srcp[base : base + n] = src[idxs]
            dstp[base : base + n] = dst[idxs]
            geo[base : base + n, 0:3] = esh[idxs]
            geo[base : base + n, 3:12] = cell[graph_batch[src[idxs]]]
            outmap[base : base + n] = idxs

        xw = np.zeros((NBLK, P, 64), np.int16)
        for b in range(NBLK):
            sb = srcp[b * BLK : (b + 1) * BLK]
            db = dstp[b * BLK : (b + 1) * BLK]
            xw[b, :, 0:32] = _wrap16(sb >> 1)
            xw[b, :, 32:64] = _wrap16(db >> 1)

        m = dict(common)
        m["xw16"] = xw
        m["geo12"] = geo
        in_maps.append(m)
        outmaps.append(outmap)
    return in_maps, outmaps, zero_bias


def _gather_out(res, outmaps, b2_host=0.0):
    full = np.empty((N_EDGES,), np.float32)
    for c in range(NCORES):
        dev = np.asarray(res.results[c]["out"])
        outmap = outmaps[c]
        valid = outmap >= 0
        full[c * E_CORE + outmap[valid]] = dev[valid]
    return full.reshape(N_EDGES, 1)


def kernel(**inputs) -> np.ndarray:
    in_maps, outmaps, zero_bias = _prep(inputs)
    nc = _get_compiled(zero_bias)
    res = run_bass_kernel_spmd(nc, in_maps, core_ids=list(range(NCORES)))
    return _gather_out(res, outmaps)


# revision 40
# speedup vs baseline: 1.1341x; 1.0193x over previous
"""Trainium2 Bass kernel for nn_ExchangeBlock (GNN message passing / e3nn-style
tensor-product edge block), SPMD across 8 NeuronCores.

Sharding: edges across the 8 cores; node features and params replicated.

Design notes:
- Edges are host-sorted into 4 parity classes (src&1, dst&1) so the pair-row
  parity select becomes a compile-time AP slice: no masks, no predicated
  copies, no gpsimd copy traffic at all.
- ONE dma_gather per block: the node table is bf16 pair rows (512B) with the
  fp32 position bit-packed into units 120:126 of each row, so geometry and
  the tensor product share the same gathered tile.  cell[graph_batch[src]]
  is a host-side index prep (like the baseline's graph_batch[src]) and is
  streamed per edge together with edge_shift.
- Blocks run in groups of 13 with two ScalarE activation-table phases per
  group (exp set: RBF; silu set: MLPs).  The per-edge scalar chain (Newton
  rsqrt, cutoff envelope as an exact degree-6 polynomial in d^2, RBF, demb)
  is batched once per group, so phase A is a handful of wide ops and the
  Square/Exp are single instructions the OoO scheduler cannot shred.
- The TP runs as outer-product features built on DVE (bf16) in two tiles
  (0e block / 1o+2e block) so the PE transpose+matmul pipeline of the 0e
  chunks overlaps the 1o/2e build; the transpose PSUM ring is 3 banks deep
  (18 touches per block).  The 1o/2e paths build m-major product tiles and
  reduce with contiguous bf16 adds instead of a slow innermost-3 reduce.
- LN stats via one PSUM reduce + a batched ScalarE Square + one bf16
  reduce; the final w2 contraction fuses multiply+reduce into one
  scalar_tensor_tensor with accum_out.
- All MLP biases in this problem are exactly zero; _prep detects that and
  compiles the bias-free variant (rank-1 PE bias matmuls otherwise).
"""

import sys

sys.path.insert(0, "/opt/trn_rl_repo")

import numpy as np
import ml_dtypes

import concourse.bass as bass
import concourse.mybir as mybir
import concourse.tile as tile
from concourse import bacc
from concourse.bass_utils import run_bass_kernel_spmd
from concourse.masks import make_identity

F32 = mybir.dt.float32
BF16 = mybir.dt.bfloat16
I32 = mybir.dt.int32
I16 = mybir.dt.int16
AF = mybir.ActivationFunctionType
OP = mybir.AluOpType

# Problem constants
L0, L1, L2 = 32, 16, 8
NS = 128
NB = 64
CUTOFF = 7.0
N_NODES = 50000
N_EDGES = 400000
NODE_DIM = 120
NCORES = 8

BLK = 512             # edges per block
SUB = 4               # 128-edge sub-tiles per block
P = 128
KTP = 1344            # 1024 + 256 + 64 contraction size
KPAD = 1408           # padded to 11 chunks of 128
NCHUNK = 11
RSQRT_MAGIC = 0x5F3759DF
NPAIR = N_NODES // 2  # 25000
XR = 128              # bf16 units per node row (120 nodes + 6 pos-halves + 2 pad)

E_CORE = N_EDGES // NCORES                      # 50000
ECLS = 13312                                    # padded edges per parity class
NBLK_CLS = ECLS // BLK                          # 26
NBLK = 4 * NBLK_CLS                             # 104
E_PAD = NBLK * BLK                              # 53248
GROUP = 13                                      # blocks per act-table phase group
GS = GROUP * SUB                                # 32 sub-tiles per group

# cos(pi/2 * sqrt(t)) Taylor coefficients, t = min(d^2/49, 1)
ENV_A = (
    1.0,
    -1.2337005500358182,
    0.25366950654487275,
    -0.020863473217859734,
    0.0009192394784838294,
    -2.5171984603292395e-05,
    4.492184960014096e-07,
)

_compiled = {}


def _patch_walrus_dge_levels():
    """This walrus build compiles with DynamicDMA disabled by default, which
    makes dynamic-offset DMAs crash the exec unit. Append the full
    --dge-levels set to every walrus invocation."""
    import concourse.bass_utils as _bu

    if getattr(_bu, "_dge_patched", False):
        return
    orig = _bu.run_command

    def patched(argv, **kw):
        if argv and "walrus_driver" in str(argv[0]) and not any(
            "dge-levels" in str(a) for a in argv
        ):
            argv = list(argv) + [
                "--dge-levels=io,spill_reload,scalar_dynamic_offset,"
                "vector_dynamic_offsets,dynamic_size,dst_reduce,transpose"
            ]
        return orig(argv, **kw)

    _bu.run_command = patched
    _bu._dge_patched = True


_patch_walrus_dge_levels()


def _patch_drain_and_barrier():
    """The final Tile drain runs on the SP engine, whose Drain lowering in this
    walrus build has no free sync-wait slots (its HWDGE queue waits fill them).
    Hoist the tile-clock waits onto dedicated nop instructions emitted just
    before the drain, one wait per nop."""
    if getattr(tile.TileContext, "_dab_patched", False):
        return

    def patched(self, tick_clock, wait_clock):
        nc = self.nc
        nops = [nc.sync.nop() for _ in range(32)]
        drain_inst = nc.sync.drain()
        from concourse.tile import ScopedClock

        wait_clock.add_sem_waits(
            drain_inst.ins, ScopedClock({None: tick_clock.global_clock})
        )
        si = drain_inst.ins.sync_info
        waits = list(si.on_wait) if si and si.on_wait else []
        if waits:
            assert len(waits) <= len(nops), f"{len(waits)} waits > nop slots"
            si.on_wait = []
            for w, n in zip(waits, nops):
                n.ins.sync_info = mybir.SyncInfo(on_wait=[w], on_update=[])

        nc.all_engine_barrier()
        assert self.sems is not None
        popped = nc._tile_sem_poison_stack.pop()
        assert popped is self._sem_poison
        nc.clear_and_free_semaphores(list(self.sems.allocated().values()))
        nc.all_engine_barrier()

    tile.TileContext._drain_and_barrier = patched
    tile.TileContext._dab_patched = True


_patch_drain_and_barrier()


def _newton_rsqrt(nc, pool, u, n, magic_t, tag, iters=3):
    """rsqrt(u) for u[:, :n] > 0 on the VectorEngine (no ScalarE table)."""
    bits = pool.tile([P, n], I32, tag=f"{tag}_b")
    nc.vector.tensor_copy(out=bits[:].bitcast(F32), in_=u)  # raw bit copy
    nc.vector.tensor_scalar(
        out=bits[:], in0=bits[:], scalar1=1, scalar2=None,
        op0=OP.arith_shift_right,
    )
    yb = pool.tile([P, n], I32, tag=f"{tag}_y")
    nc.vector.tensor_tensor(
        out=yb[:], in0=magic_t[:, 0:1].to_broadcast([P, n]), in1=bits[:],
        op=OP.subtract,
    )
    y = yb[:].bitcast(F32)
    t1 = pool.tile([P, n], F32, tag=f"{tag}_t1")
    for _ in range(iters):
        nc.vector.tensor_mul(t1[:], y, y)
        nc.vector.tensor_mul(t1[:], t1[:], u)
        nc.vector.tensor_scalar(
            out=t1[:], in0=t1[:], scalar1=-0.5, scalar2=1.5, op0=OP.mult, op1=OP.add,
        )
        nc.vector.tensor_mul(y, y, t1[:])
    return yb


def _build(nblocks: int, zero_bias: bool):
    nc = bacc.Bacc("TRN2", target_bir_lowering=False, debug=False)

    nodes_pair = nc.dram_tensor("nodes_pair", (NPAIR, 2 * XR), BF16, kind="ExternalInput").ap()
    xw16 = nc.dram_tensor("xw16", (nblocks, P, 64), I16, kind="ExternalInput").ap()
    geo12 = nc.dram_tensor("geo12", (nblocks * BLK, 12), F32, kind="ExternalInput").ap()
    wflat = nc.dram_tensor("wflat", (KPAD, NS), BF16, kind="ExternalInput").ap()
    dfw1 = nc.dram_tensor("dfw1", (NB, 128), BF16, kind="ExternalInput").ap()
    dfb1 = nc.dram_tensor("dfb1", (1, 128), BF16, kind="ExternalInput").ap()
    dfw2gb = nc.dram_tensor("dfw2gb", (128, 256), BF16, kind="ExternalInput").ap()
    dfb2gb = nc.dram_tensor("dfb2gb", (1, 256), BF16, kind="ExternalInput").ap()
    mlpw1 = nc.dram_tensor("mlpw1", (128, 512), BF16, kind="ExternalInput").ap()
    mlpb1 = nc.dram_tensor("mlpb1", (1, 512), BF16, kind="ExternalInput").ap()
    w2row = nc.dram_tensor("w2row", (1, 512), BF16, kind="ExternalInput").ap()
    b2sc = nc.dram_tensor("b2sc", (1, 1), F32, kind="ExternalInput").ap()
    offs = nc.dram_tensor("offs", (1, NB), F32, kind="ExternalInput").ap()
    out = nc.dram_tensor("out", (nblocks * BLK,), F32, kind="ExternalOutput").ap()

    width = CUTOFF / (NB - 1)
    coeff = 0.5 / (width * width)
    sqc = float(np.sqrt(coeff))

    XGBUFS = GROUP + 6

    with tile.TileContext(nc) as tc:
        with (
            tc.tile_pool(name="const", bufs=1) as constp,
            tc.tile_pool(name="xgp", bufs=XGBUFS) as xgp,
            tc.tile_pool(name="grp", bufs=3) as grpp,
            tc.tile_pool(name="rbp", bufs=2) as rbp,
            tc.tile_pool(name="io", bufs=6) as iop,
            tc.tile_pool(name="geo", bufs=5) as geop,
            tc.tile_pool(name="pfeat", bufs=3) as pfp,
            tc.tile_pool(name="trsb", bufs=4) as trsbp,
            tc.tile_pool(name="work", bufs=5) as workp,
            tc.tile_pool(name="acc", bufs=4) as accp,
            tc.tile_pool(name="ps_t8", bufs=3, space="PSUM") as ps_t8,
            tc.tile_pool(name="ps_mm", bufs=2, space="PSUM") as ps_mm,
            tc.tile_pool(name="ps_hd", bufs=1, space="PSUM") as ps_hd,
            tc.tile_pool(name="ps_g", bufs=1, space="PSUM") as ps_g,
        ):
            # ---- resident constants ----
            identb = constp.tile([P, P], BF16)
            make_identity(nc, identb[:])
            magic_t = constp.tile([P, 1], I32)
            nc.vector.memset(magic_t[:], RSQRT_MAGIC)
            ones_row = constp.tile([1, P], BF16)
            nc.vector.memset(ones_row[:], 1.0)

            w_sb = constp.tile([P, NCHUNK, P], BF16)
            nc.sync.dma_start(out=w_sb[:], in_=wflat.rearrange("(c p) w -> p c w", p=P))
            dfw1_sb = constp.tile([P, 128], BF16)
            nc.sync.dma_start(out=dfw1_sb[0:NB, :], in_=dfw1)
            nc.sync.dma_start(out=dfw1_sb[NB:P, :], in_=dfw1)
            if zero_bias:
                dfw2gb_sb = constp.tile([128, 128], BF16)
                nc.sync.dma_start(out=dfw2gb_sb[:], in_=dfw2gb[:, 0:128])
            else:
                dfw2gb_sb = constp.tile([128, 256], BF16)
                nc.sync.dma_start(out=dfw2gb_sb[:], in_=dfw2gb)
            mlpw1_sb = constp.tile([128, 512], BF16)
            nc.sync.dma_start(out=mlpw1_sb[:], in_=mlpw1)
            w2rep_sb = constp.tile([P, 512], BF16)
            nc.sync.dma_start(out=w2rep_sb[:], in_=w2row.to_broadcast([P, 512]))
            b2_sb = constp.tile([P, 1], F32)
            nc.sync.dma_start(out=b2_sb[:], in_=b2sc.to_broadcast([P, 1]))
            offs_sb = constp.tile([P, NB], F32)
            nc.sync.dma_start(out=offs_sb[:], in_=offs.to_broadcast([P, NB]))
            if not zero_bias:
                dfb1_sb = constp.tile([1, 128], BF16)
                nc.sync.dma_start(out=dfb1_sb[:], in_=dfb1)
                dfb2gb_sb = constp.tile([1, 256], BF16)
                nc.sync.dma_start(out=dfb2gb_sb[:], in_=dfb2gb)
                mlpb1_sb = constp.tile([1, 512], BF16)
                nc.sync.dma_start(out=mlpb1_sb[:], in_=mlpb1)

            xg_tiles = {}

            groups = [range(g, min(g + GROUP, nblocks)) for g in range(0, nblocks, GROUP)]
            for blocks in groups:
                gn = len(blocks)
                g0 = blocks[0]
                # ======== Phase A: gather + geometry + RBF (exp table) ========
                d2g = grpp.tile([P, GS], F32, tag="d2g")
                geog = grpp.tile([P, GROUP, SUB, 12], F32, tag="geog")
                nc.sync.dma_start(
                    out=geog[:, 0:gn, :, :],
                    in_=geo12[g0 * BLK : (g0 + gn) * BLK, :].rearrange(
                        "(g s p) j -> p g s j", p=P, s=SUB
                    ),
                )
                tvp = grpp.tile([P, GROUP, SUB, 3, 3], F32, tag="tvp")
                nc.vector.tensor_tensor(
                    out=tvp[:, 0:gn],
                    in0=geog[:, 0:gn, :, 0:3].unsqueeze(4).to_broadcast([P, gn, SUB, 3, 3]),
                    in1=geog[:, 0:gn, :, 3:12].rearrange("p g s (i j) -> p g s i j", j=3),
                    op=OP.mult,
                )
                tvg = grpp.tile([P, GROUP, SUB, 3], F32, tag="tvg")
                nc.vector.reduce_sum(
                    out=tvg[:, 0:gn], in_=tvp[:, 0:gn].transpose([0, 1, 2, 4, 3]),
                    axis=mybir.AxisListType.X,
                )
                for i, b in enumerate(blocks):
                    cls = b // NBLK_CLS
                    ps, pd = (cls >> 1) & 1, cls & 1

                    xw = iop.tile([P, 64], I16, tag="xw")
                    nc.sync.dma_start(out=xw[:], in_=xw16[b])
                    xg = xgp.tile([P, 2 * SUB, 2 * XR], BF16, tag="xg")
                    nc.gpsimd.dma_gather(
                        out_ap=xg[:], in_ap=nodes_pair[:, :], idxs_ap=xw[:],
                        num_idxs=2 * BLK, num_idxs_reg=2 * BLK, elem_size=2 * XR,
                    )
                    xg_tiles[b] = xg

                    # fp32 positions bit-packed into the bf16 rows
                    p1 = xg[:, 0:SUB, ps * XR + 120 : ps * XR + 126].bitcast(F32)
                    p2 = xg[:, SUB : 2 * SUB, pd * XR + 120 : pd * XR + 126].bitcast(F32)

                    rv = geop.tile([P, SUB, 3], F32, tag="rv")
                    nc.vector.tensor_sub(rv[:], p2, p1)
                    nc.vector.tensor_add(rv[:], rv[:], tvg[:, i])
                    rv2 = geop.tile([P, SUB, 3], F32, tag="rv2")
                    nc.vector.tensor_mul(rv2[:], rv[:], rv[:])
                    nc.vector.reduce_sum(
                        out=d2g[:, i * SUB : (i + 1) * SUB], in_=rv2[:],
                        axis=mybir.AxisListType.X,
                    )

                ng = gn * SUB
                nc.vector.tensor_scalar(
                    out=d2g[:, 0:ng], in0=d2g[:, 0:ng], scalar1=1e-12, scalar2=None,
                    op0=OP.max,
                )
                ry = _newton_rsqrt(nc, grpp, d2g[:, 0:ng], ng, magic_t, "rsq", iters=2)
                dist = grpp.tile([P, GS], F32, tag="dist")
                nc.vector.tensor_mul(dist[:, 0:ng], d2g[:, 0:ng], ry[:].bitcast(F32))

                # envelope: env = p(t)^2, t = min(d2/49, 1)
                tgeo = grpp.tile([P, GS], F32, tag="tgeo")
                nc.vector.tensor_scalar(
                    out=tgeo[:, 0:ng], in0=d2g[:, 0:ng], scalar1=1.0 / 49.0, scalar2=1.0,
                    op0=OP.mult, op1=OP.min,
                )
                envr = grpp.tile([P, GS], F32, tag="envr")
                nc.vector.tensor_scalar(
                    out=envr[:, 0:ng], in0=tgeo[:, 0:ng], scalar1=ENV_A[6], scalar2=None,
                    op0=OP.mult,
                )
                for k in range(5, 0, -1):
                    nc.vector.scalar_tensor_tensor(
                        out=envr[:, 0:ng], in0=envr[:, 0:ng], scalar=ENV_A[k],
                        in1=tgeo[:, 0:ng], op0=OP.add, op1=OP.mult,
                    )
                env = grpp.tile([P, GS], F32, tag="env")
                nc.vector.tensor_scalar(
                    out=env[:, 0:ng], in0=envr[:, 0:ng], scalar1=ENV_A[0], scalar2=None,
                    op0=OP.add,
                )
                nc.vector.tensor_mul(env[:, 0:ng], env[:, 0:ng], env[:, 0:ng])

                # rbf then demb = rbf * env (one Square + one Exp per group)
                rb = rbp.tile([P, GS, NB], F32, tag="rb")
                nc.vector.tensor_tensor(
                    out=rb[:, 0:ng, :],
                    in0=offs_sb[:].unsqueeze(1).to_broadcast([P, ng, NB]),
                    in1=dist[:, 0:ng].unsqueeze(2).to_broadcast([P, ng, NB]),
                    op=OP.subtract,
                )
                nc.scalar.activation(rb[:, 0:ng, :], rb[:, 0:ng, :], AF.Square, scale=sqc)
                demb = grpp.tile([P, GS, NB], BF16, tag="demb")
                if zero_bias:
                    # env is folded into the dfilter silu scale downstream
                    nc.scalar.activation(demb[:, 0:ng, :], rb[:, 0:ng, :], AF.Exp, scale=-1.0)
                else:
                    nc.scalar.activation(rb[:, 0:ng, :], rb[:, 0:ng, :], AF.Exp, scale=-1.0)
                    nc.vector.tensor_tensor(
                        out=demb[:, 0:ng, :], in0=rb[:, 0:ng, :],
                        in1=env[:, 0:ng].unsqueeze(2).to_broadcast([P, ng, NB]),
                        op=OP.mult,
                    )

                # ======== Phase B: TP + LN + dfilter + MLP (silu table) ========
                for j0 in range(0, gn, 2):
                  pair = list(blocks[j0 : j0 + 2])
                  np_pair = len(pair)
                  sumv = geop.tile([P, 2, SUB], F32, tag="sumv")
                  sumsq = geop.tile([P, 2, SUB], F32, tag="sumsq")
                  psmixes = {}
                  for jj, b in enumerate(pair):
                    i = j0 + jj
                    cls = b // NBLK_CLS
                    ps, pd = (cls >> 1) & 1, cls & 1
                    xg = xg_tiles.pop(b)

                    x1 = xg[:, 0:SUB, ps * XR : ps * XR + 120]
                    x2 = xg[:, SUB : 2 * SUB, pd * XR : pd * XR + 120]

                    psmix = ps_mm.tile([P, SUB, NS], F32, tag="psmix")
                    psmixes[b] = psmix

                    # ---- pass 1: tensor product per sub-tile ----
                    for s in range(SUB):
                        ptb0 = pfp.tile([P, 1024], BF16, tag="ptb0")
                        ptb1 = pfp.tile([P, 384], BF16, tag="ptb1")
                        a1 = x1[:, s, 0:L0]
                        a2 = x2[:, s, 0:L0]
                        nc.vector.tensor_tensor(
                            out=ptb0[:].rearrange("p (u v) -> p u v", v=L0),
                            in0=a1.unsqueeze(2).to_broadcast([P, L0, L0]),
                            in1=a2.unsqueeze(1).to_broadcast([P, L0, L0]),
                            op=OP.mult,
                        )
                        # 0e transposes + copy + matmuls depend only on ptb0,
                        # overlapping the 1o/2e build below
                        ptp8 = ps_t8.tile([P, 8, P], BF16, tag="ptp8")
                        for c in range(8):
                            nc.tensor.transpose(
                                ptp8[:, c, :], ptb0[:, c * P : (c + 1) * P], identb[:]
                            )
                        pts8 = trsbp.tile([P, 8, P], BF16, tag="pts8")
                        nc.scalar.copy(pts8[:], ptp8[:])
                        for c in range(8):
                            nc.tensor.matmul(
                                psmix[:, s, :], lhsT=pts8[:, c, :], rhs=w_sb[:, c, :],
                                start=(c == 0), stop=False,
                            )
                        # 1o path: m-major product tile, reduce via 2 contiguous adds
                        b1 = x1[:, s, 32:80].rearrange("p (u m) -> p m u", m=3)
                        b2 = x2[:, s, 32:80].rearrange("p (v m) -> p m v", m=3)
                        pb = workp.tile([P, 3, L1, L1], BF16, tag="pb")
                        nc.vector.tensor_tensor(
                            out=pb[:],
                            in0=b1.unsqueeze(3).to_broadcast([P, 3, L1, L1]),
                            in1=b2.unsqueeze(2).to_broadcast([P, 3, L1, L1]),
                            op=OP.mult,
                        )
                        pbf = pb[:].rearrange("p m u v -> p m (u v)")
                        with nc.allow_low_precision(reason="3-term bf16 add"):
                            t01 = workp.tile([P, L1 * L1], BF16, tag="t01")
                            nc.vector.tensor_add(t01[:], pbf[:, 0, :], pbf[:, 1, :])
                            nc.vector.tensor_add(ptb1[:, 0:256], t01[:], pbf[:, 2, :])
                        # 2e path
                        c1 = x1[:, s, 80:120].rearrange("p (u m) -> p m u", m=5)
                        c2 = x2[:, s, 80:120].rearrange("p (v m) -> p m v", m=5)
                        pc = workp.tile([P, 5, L2, L2], BF16, tag="pc")
                        nc.vector.tensor_tensor(
                            out=pc[:],
                            in0=c1.unsqueeze(3).to_broadcast([P, 5, L2, L2]),
                            in1=c2.unsqueeze(2).to_broadcast([P, 5, L2, L2]),
                            op=OP.mult,
                        )
                        pcf = pc[:].rearrange("p m u v -> p m (u v)")
                        with nc.allow_low_precision(reason="5-term bf16 add"):
                            u01 = workp.tile([P, L2 * L2], BF16, tag="u01")
                            nc.vector.tensor_add(u01[:], pcf[:, 0, :], pcf[:, 1, :])
                            u23 = workp.tile([P, L2 * L2], BF16, tag="u23")
                            nc.vector.tensor_add(u23[:], pcf[:, 2, :], pcf[:, 3, :])
                            nc.vector.tensor_add(u01[:], u01[:], u23[:])
                            nc.vector.tensor_add(ptb1[:, 256:320], u01[:], pcf[:, 4, :])

                        ptp4 = ps_t8.tile([P, 8, P], BF16, tag="ptp8")
                        nc.tensor.transpose(ptp4[:, 0, :], ptb1[:, 0:128], identb[:])
                        nc.tensor.transpose(ptp4[:, 1, :], ptb1[:, 128:256], identb[:])
                        nc.tensor.transpose(ptp4[0:64, 2, :], ptb1[:, 256:320], identb[:])
                        pts4 = trsbp.tile([P, 4, P], BF16, tag="pts4")
                        nc.scalar.copy(pts4[:, 0:3, :], ptp4[:, 0:3, :])
                        nc.tensor.matmul(
                            psmix[:, s, :], lhsT=pts4[:, 0, :], rhs=w_sb[:, 8, :],
                            start=False, stop=False,
                        )
                        nc.tensor.matmul(
                            psmix[:, s, :], lhsT=pts4[:, 1, :], rhs=w_sb[:, 9, :],
                            start=False, stop=False,
                        )
                        nc.tensor.matmul(
                            psmix[:, s, :], lhsT=pts4[0:64, 2, :], rhs=w_sb[0:64, 10, :],
                            start=False, stop=True,
                        )

                    # per-block LN stat reduces into the pair tile
                    nc.vector.reduce_sum(
                        out=sumv[:, jj, :], in_=psmix[:], axis=mybir.AxisListType.X,
                    )
                    sq = workp.tile([P, SUB, NS], BF16, tag="sq")
                    nc.scalar.activation(sq[:], psmix[:], AF.Square)
                    with nc.allow_low_precision(reason="bf16 sumsq reduce"):
                        nc.vector.reduce_sum(
                            out=sumsq[:, jj, :], in_=sq[:], axis=mybir.AxisListType.X,
                        )

                  # ---- pair-level LN scalar chain ----
                  nsx = np_pair * SUB
                  sumvf = sumv[:].rearrange("p j s -> p (j s)")
                  sumsqf = sumsq[:].rearrange("p j s -> p (j s)")
                  muv = geop.tile([P, 2 * SUB], F32, tag="muv")
                  nc.vector.tensor_scalar(
                      out=muv[:, 0:nsx], in0=sumvf[:, 0:nsx], scalar1=1.0 / NS,
                      scalar2=None, op0=OP.mult,
                  )
                  varv = geop.tile([P, 2 * SUB], F32, tag="varv")
                  nc.vector.tensor_mul(varv[:, 0:nsx], muv[:, 0:nsx], muv[:, 0:nsx])
                  nc.vector.scalar_tensor_tensor(
                      out=varv[:, 0:nsx], in0=sumsqf[:, 0:nsx], scalar=1.0 / NS,
                      in1=varv[:, 0:nsx], op0=OP.mult, op1=OP.subtract,
                  )
                  nc.vector.tensor_scalar(
                      out=varv[:, 0:nsx], in0=varv[:, 0:nsx], scalar1=1e-5,
                      scalar2=None, op0=OP.add,
                  )
                  ryl = _newton_rsqrt(nc, geop, varv[:, 0:nsx], nsx, magic_t, "lnr", iters=2)
                  rstd_all = ryl[:].bitcast(F32)
                  tb_all = geop.tile([P, 2 * SUB], F32, tag="tb")
                  nc.vector.scalar_tensor_tensor(
                      out=tb_all[:, 0:nsx], in0=muv[:, 0:nsx], scalar=-1.0,
                      in1=rstd_all, op0=OP.mult, op1=OP.mult,
                  )

                  for jj, b in enumerate(pair):
                    i = j0 + jj
                    cls = b // NBLK_CLS
                    ps, pd = (cls >> 1) & 1, cls & 1
                    e0 = b * BLK
                    sl = slice(e0, e0 + BLK)
                    psmix = psmixes[b]
                    rstd = rstd_all[:, jj * SUB : (jj + 1) * SUB]
                    tb = tb_all[:, jj * SUB : (jj + 1) * SUB]

                    acc = accp.tile([P, SUB], F32, tag="acc")

                    # batched dT transposes: two sub-tiles of demb per transpose
                    dTs = []
                    for h in range(2):
                        dT_ps = ps_t8.tile([P, 8, P], BF16, tag="ptp8")
                        nc.tensor.transpose(
                            dT_ps[:, 0, :],
                            demb[:, (i * SUB + 2 * h) : (i * SUB + 2 * h + 2), :]
                            .rearrange("p s k -> p (s k)"),
                            identb[:],
                        )
                        dT = trsbp.tile([P, P], BF16, tag=f"dT{h}")
                        nc.scalar.copy(dT[:], dT_ps[:, 0, :])
                        dTs.append(dT)

                    # ---- LN apply for all sub-tiles up front (frees psmix) ----
                    ynorms = []
                    for s in range(SUB):
                        ynorm = workp.tile([P, NS], BF16, tag="ynorm")
                        nc.scalar.activation(
                            ynorm[:], psmix[:, s, :], AF.Identity,
                            bias=tb[:, s : s + 1], scale=rstd[:, s : s + 1],
                        )
                        ynorms.append(ynorm)

                    # ---- pass 2: dfilter + final MLP ----
                    for s in range(SUB):
                        ynorm = ynorms[s]
                        h0 = (s % 2) * NB
                        dT = dTs[s // 2][h0 : h0 + NB, :]
                        rhs1 = dfw1_sb[h0 : h0 + NB, :]
                        ph = ps_hd.tile([P, 128], F32, tag="ph")
                        if zero_bias:
                            nc.tensor.matmul(ph[:], lhsT=dT, rhs=rhs1, start=True, stop=True)
                        else:
                            nc.tensor.matmul(ph[:], lhsT=dT, rhs=rhs1, start=True, stop=False)
                            nc.tensor.matmul(ph[:], lhsT=ones_row[:], rhs=dfb1_sb[:], start=False, stop=True)
                        sact = workp.tile([P, 128], BF16, tag="sact")
                        if zero_bias:
                            nc.scalar.activation(
                                sact[:], ph[:], AF.Silu,
                                scale=env[:, i * SUB + s : i * SUB + s + 1],
                            )
                        else:
                            nc.scalar.activation(sact[:], ph[:], AF.Silu)
                        sT_ps = ps_t8.tile([P, 8, P], BF16, tag="ptp8")
                        nc.tensor.transpose(sT_ps[:, 0, :], sact[:], identb[:])
                        sT = trsbp.tile([P, P], BF16, tag="sT")
                        nc.vector.tensor_copy(sT[:], sT_ps[:, 0, :])
                        rg = workp.tile([P, 128], BF16, tag="rg")
                        if zero_bias:
                            pdf = ps_hd.tile([P, 128], F32, tag="pdf")
                            nc.tensor.matmul(pdf[:], lhsT=sT[:], rhs=dfw2gb_sb[:], start=True, stop=True)
                            nc.vector.tensor_mul(rg[:], ynorm[:], pdf[:])
                        else:
                            pdf = ps_hd.tile([P, 256], F32, tag="pdf")
                            nc.tensor.matmul(pdf[:], lhsT=sT[:], rhs=dfw2gb_sb[:], start=True, stop=False)
                            nc.tensor.matmul(pdf[:], lhsT=ones_row[:], rhs=dfb2gb_sb[:], start=False, stop=True)
                            nc.vector.tensor_mul(rg[:], ynorm[:], pdf[:, 0:128])
                            nc.vector.tensor_add(rg[:], rg[:], pdf[:, 128:256])

                        rT_ps = ps_t8.tile([P, 8, P], BF16, tag="ptp8")
                        nc.tensor.transpose(rT_ps[:, 0, :], rg[:], identb[:])
                        rT = trsbp.tile([P, P], BF16, tag="rT")
                        nc.scalar.copy(rT[:], rT_ps[:, 0, :])
                        pg2 = ps_g.tile([P, 512], F32, tag="pg")
                        if zero_bias:
                            nc.tensor.matmul(pg2[:], lhsT=rT[:], rhs=mlpw1_sb[:], start=True, stop=True)
                        else:
                            nc.tensor.matmul(pg2[:], lhsT=rT[:], rhs=mlpw1_sb[:], start=True, stop=False)
                            nc.tensor.matmul(pg2[:], lhsT=ones_row[:], rhs=mlpb1_sb[:], start=False, stop=True)
                        gact = workp.tile([P, 512], BF16, tag="gact")
                        nc.scalar.activation(gact[:], pg2[:], AF.Silu)
                        scr = workp.tile([P, 512], BF16, tag="scr")
                        nc.vector.scalar_tensor_tensor(
                            out=scr[:], in0=gact[:], scalar=1.0, in1=w2rep_sb[:],
                            op0=OP.mult, op1=OP.mult,
                            accum_out=acc[:, s : s + 1],
                        )

                    if not zero_bias:
                        nc.vector.tensor_scalar(
                            out=acc[:], in0=acc[:], scalar1=b2_sb[:, 0:1], scalar2=None,
                            op0=OP.add,
                        )
                    nc.sync.dma_start(out=out[sl].rearrange("(s p) -> p s", p=P), in_=acc[:])

    nc.compile()
    return nc


def _get_compiled(zero_bias: bool = True):
    if zero_bias not in _compiled:
        _compiled[zero_bias] = _build(NBLK, zero_bias)
    return _compiled[zero_bias]


def _wrap16(idx_block):
    """int array [512] -> dma_gather wrapped int16 layout [128, 32]
    (index j at [j%16, j//16], replicated across the 8 gpsimd cores)."""
    w = idx_block.astype(np.int16).reshape(-1, 16).T  # [16, n/16]
    return np.tile(w, (8, 1))


def _prep(inputs):
    nodes = np.asarray(inputs["nodes"], np.float32)
    edge_index = np.asarray(inputs["edge_index"]).astype(np.int64)
    graph_batch = np.asarray(inputs["graph_batch"]).astype(np.int64)
    cell = np.asarray(inputs["cell"], np.float32).reshape(32, 9)
    edge_shift = np.asarray(inputs["edge_shift"], np.float32)
    pos = np.asarray(inputs["pos"], np.float32)

    # bf16 pair-row node table with fp32 pos bit-packed at units 120:126
    row_u16 = np.zeros((N_NODES, XR), np.uint16)
    row_u16[:, 0:NODE_DIM] = nodes.astype(ml_dtypes.bfloat16).view(np.uint16)
    row_u16[:, 120:126] = pos.view(np.uint16).reshape(N_NODES, 6)
    nodes_pair = row_u16.reshape(NPAIR, 2 * XR).view(ml_dtypes.bfloat16)

    alpha = 1.0 / np.sqrt(float(L0 * L0 + L1 * L1 + L2 * L2))
    w0 = np.asarray(inputs["W0"], np.float32).reshape(L0 * L0, NS) * alpha
    w1 = np.asarray(inputs["W1"], np.float32).reshape(L1 * L1, NS) * (alpha / np.sqrt(3.0))
    w2 = np.asarray(inputs["W2"], np.float32).reshape(L2 * L2, NS) * (alpha / np.sqrt(5.0))
    wflat = np.zeros((KPAD, NS), np.float32)
    wflat[0:1024] = w0
    wflat[1024:1280] = w1
    wflat[1280:1344] = w2

    ln_g = np.asarray(inputs["ln_g"], np.float32)
    ln_b = np.asarray(inputs["ln_b"], np.float32)
    df_w2 = np.asarray(inputs["df_w2"], np.float32)
    df_b2 = np.asarray(inputs["df_b2"], np.float32)
    dfw2gb = np.concatenate([df_w2 * ln_g[None, :], df_w2 * ln_b[None, :]], axis=1)
    dfb2gb = np.concatenate([df_b2 * ln_g, df_b2 * ln_b])[None, :]

    zero_bias = (
        not np.any(np.asarray(inputs["df_b1"]))
        and not np.any(np.asarray(inputs["df_b2"]))
        and not np.any(np.asarray(inputs["mlp_b1"]))
        and not np.any(np.asarray(inputs["mlp_b2"]))
        and not np.any(ln_b)
    )

    bf = lambda a: np.ascontiguousarray(a).astype(ml_dtypes.bfloat16)

    common = {
        "nodes_pair": nodes_pair,
        "wflat": bf(wflat),
        "dfw1": bf(np.asarray(inputs["df_w1"], np.float32)),
        "dfb1": bf(np.asarray(inputs["df_b1"], np.float32)[None, :]),
        "dfw2gb": bf(dfw2gb),
        "dfb2gb": bf(dfb2gb),
        "mlpw1": bf(np.asarray(inputs["mlp_w1"], np.float32)),
        "mlpb1": bf(np.asarray(inputs["mlp_b1"], np.float32)[None, :]),
        "w2row": bf(np.asarray(inputs["mlp_w2"], np.float32).T),
        "b2sc": np.asarray(inputs["mlp_b2"], np.float32).reshape(1, 1),
        "offs": np.linspace(0.0, CUTOFF, NB, dtype=np.float32)[None, :],
    }

    in_maps = []
    outmaps = []
    for c in range(NCORES):
        lo, hi = c * E_CORE, (c + 1) * E_CORE
        src = edge_index[0, lo:hi]
        dst = edge_index[1, lo:hi]
        esh = edge_shift[lo:hi]
        key = ((src & 1) << 1) | (dst & 1)

        srcp = np.zeros(E_PAD, np.int64)
        dstp = np.zeros(E_PAD, np.int64)
        geo = np.zeros((E_PAD, 12), np.float32)
        outmap = np.full(E_PAD, -1, np.int64)
        for cls in range(4):
            idxs = np.nonzero(key == cls)[0]
            n = len(idxs)
            assert n <= ECLS, f"class {cls} overflow: {n} > {ECLS}"
            base = cls * ECLS
            srcp[base : base + n] = src[idxs]
            dstp[base : base + n] = dst[idxs]
            geo[base : base + n, 0:3] = esh[idxs]
            geo[base : base + n, 3:12] = cell[graph_batch[src[idxs]]]
            outmap[base : base + n] = idxs

        xw = np.zeros((NBLK, P, 64), np.int16)
        for b in range(NBLK):
            sb = srcp[b * BLK : (b + 1) * BLK]
            db = dstp[b * BLK : (b + 1) * BLK]
            xw[b, :, 0:32] = _wrap16(sb >> 1)
            xw[b, :, 32:64] = _wrap16(db >> 1)

        m = dict(common)
        m["xw16"] = xw
        m["geo12"] = geo
        in_maps.append(m)
        outmaps.append(outmap)
    return in_maps, outmaps, zero_bias


def _gather_out(res, outmaps, b2_host=0.0):
    full = np.empty((N_EDGES,), np.float32)
    for c in range(NCORES):
        dev = np.asarray(res.results[c]["out"])
        outmap = outmaps[c]
        valid = outmap >= 0
        full[c * E_CORE + outmap[valid]] = dev[valid]
    return full.reshape(N_EDGES, 1)


def kernel(**inputs) -> np.ndarray:
    in_maps, outmaps, zero_bias = _prep(inputs)
    nc = _get_compiled(zero_bias)
    res = run_bass_kernel_spmd(nc, in_maps, core_ids=list(range(NCORES)))
    return _gather_out(res, outmaps)
